# revision 11
# baseline (speedup 1.0000x reference)
"""CFNet interaction block on 8 trn2 NeuronCores (SPMD bass/tile kernel), v2.

Per core c of 8 (SPMD, one program, per-core data): core c owns atoms
[c*NA, (c+1)*NA) and the edges whose sorted seg_i lands there.

Host prep = pure layout (no reference FLOPs):
  - dijk cast fp32->bf16 and pre-TRANSPOSED into k-chunks [128|128|44, E_PC]
    (kills the device-side PE transposes and halves dijk HBM traffic),
  - x rows pre-gathered by idx_j, transposed: xg^T [128x, E_PC] bf16
    (kills the v1 per-edge dma_gather: ~9 ns of Q7 per edge),
  - one-hot S pages [T, 128, 128] bf16: edge row -> column (atom mod 128).

Static SPMD schedule: local atoms are split into 128-atom chunks; chunk k
gets a FIXED tile allotment TPW[k] (mean + 6 sigma), so every core's tile t
maps to the same chunk and the same psum window -- per-core variation is
absorbed by padding (~7% edge inflation).  Pad edges carry all-zero S rows.

Device pipeline per 512-edge block, [feature, edge] layout (weights are the
matmul stationaries):
  mm1  t1 = W1c.T @ dT (3 k-chunks)          psum [f1, e]
  ssp1 Exp (psum->sbuf, per block), Ln(0.5x+0.5) batched [128, 2048] -> bf16
  mm2  w^T = W2.T @ t1s                      psum [f2, e]
  sspw Exp per block, Ln batched             -> wt^T fp32
  mm_f f^T = Win.T @ xg^T                    psum [f, e]
  wf^T = wt^T * f^T (DVE)                    -> bf16
  PE-transpose wf^T -> wf [e, f] (psum bf16 -> sbuf)
  mm3 per 128-edge tile: conv^T[chunk] += wf_tile.T @ S_tile
       into a [128, 128] psum window per chunk; ~49 flushes to sbuf.
A single manual InstLoadActFuncSet(natural_log_exp_and_others) keeps Exp+Ln
resident: no ACT_TABLE_LOAD thrash (was 2.6 ms in v1).

Tail from sbuf-resident conv^T: z3^T = Wout.T @ conv^T, ssp, v^T = Wd.T @
h^T, y^T = v^T + x^T.  Outputs leave TRANSPOSED [128, NA_PAD]; the host
transposes back.  ssp(x) = Ln(0.5 + 0.5*Exp(x)) exactly.
"""

import math
import sys

import numpy as np
import ml_dtypes

sys.path.insert(0, "/opt/trn_rl_repo")

import concourse.bacc as bacc
import concourse.mybir as mybir
from concourse import tile
from concourse.bass_utils import run_bass_kernel_spmd

dt = mybir.dt
AF = mybir.ActivationFunctionType
BF16 = ml_dtypes.bfloat16

N_CORES = 8
TILE_E = 128            # edges per S tile / conv matmul
BLK = 512               # edges per pipeline block
GRP = 2048              # edges per DMA group (4 blocks, 16 tiles)
CHUNK_A = 128           # atoms per conv psum window
ACT_SET_LN_EXP = 6      # natural_log_exp_and_others in act_info.json


def _ceil(a, b):
    return -(-a // b)


def _to_bf16(a):
    """fp32 -> bf16 with round-to-nearest-even, fast numpy path."""
    a = np.ascontiguousarray(a, dtype=np.float32)
    v = a.view(np.uint32)
    r = ((v + np.uint32(0x7FFF) + ((v >> np.uint32(16)) & np.uint32(1)))
         >> np.uint32(16)).astype(np.uint16)
    return r.view(BF16).reshape(a.shape)


class Plan:
    """Structure constants; identical for every core.  The per-chunk tile
    allotment TPW is fitted to the ACTUAL seg_i data (max over cores), so
    padding is just tile rounding (~3%) and fits by construction."""

    def __init__(self, n_atoms, n_edges, n_in, seg_i):
        assert n_atoms % N_CORES == 0
        self.n_atoms, self.n_edges, self.n_in = n_atoms, n_edges, n_in
        self.NA = n_atoms // N_CORES
        self.NCHUNK_REAL = _ceil(self.NA, CHUNK_A)
        seg_i = np.asarray(seg_i).astype(np.int64)
        bounds = np.searchsorted(seg_i, np.arange(N_CORES + 1) * self.NA)
        mx = np.zeros(self.NCHUNK_REAL, dtype=np.int64)
        for c in range(N_CORES):
            es = seg_i[bounds[c]:bounds[c + 1]] - c * self.NA
            cnt = np.bincount(es // CHUNK_A, minlength=self.NCHUNK_REAL)
            mx = np.maximum(mx, cnt)
        tpw = [int(_ceil(int(m), TILE_E)) for m in mx]
        t_raw = sum(tpw)
        self.T = int(_ceil(t_raw, GRP // TILE_E) * (GRP // TILE_E))
        self.TPW = tpw
        self.E_PC = self.T * TILE_E
        self.NBLK = self.E_PC // BLK
        self.NGRP = self.E_PC // GRP
        self.KC = [min(128, n_in - 128 * i) for i in range(_ceil(n_in, 128))]
        self.NKC = len(self.KC)
        # tile -> chunk map; trailing pad tiles attach to the last chunk
        self.first_tile = []
        self.tile_chunk = []
        for k, n in enumerate(tpw):
            self.first_tile.append(len(self.tile_chunk))
            self.tile_chunk += [k] * n
        self.tile_chunk += [self.NCHUNK_REAL - 1] * (self.T - t_raw)
        self.last_tile = [0] * self.NCHUNK_REAL
        for t, k in enumerate(self.tile_chunk):
            self.last_tile[k] = t
        self.NA_PAD = self.NCHUNK_REAL * CHUNK_A
        self.NSLAB = _ceil(self.NA_PAD, 512)         # tail slabs of 512 atoms
        self.NA_TAIL = self.NSLAB * 512


def shard_inputs(p, x, dijk_bf_T, xgT_all, seg_i):
    """Per-core layout prep. dijk_bf_T/xgT_all carry a zero pad column at
    index n_edges."""
    seg_i = np.asarray(seg_i).astype(np.int64)
    bounds = np.searchsorted(seg_i, np.arange(N_CORES + 1) * p.NA)
    ZCOL = p.n_edges                                  # the zero column

    per_core = []
    for c in range(N_CORES):
        lo, hi = int(bounds[c]), int(bounds[c + 1])
        es = seg_i[lo:hi] - c * p.NA                  # local atoms, sorted
        chunk = es // CHUNK_A

        # per-chunk edge placement into the static tile schedule
        cols = np.full(p.E_PC, ZCOL, dtype=np.int64)  # global edge id or pad
        s_t = np.empty(hi - lo, dtype=np.int64)       # tile of each edge
        s_r = np.empty(hi - lo, dtype=np.int64)       # row within tile
        cnt = np.bincount(chunk, minlength=p.NCHUNK_REAL)
        for k in range(p.NCHUNK_REAL):
            n_k = int(cnt[k])
            if n_k == 0:
                continue
            assert n_k <= p.TPW[k] * TILE_E, (c, k, n_k, p.TPW[k] * TILE_E)
            e0 = int(np.searchsorted(chunk, k))
            base = p.first_tile[k] * TILE_E
            pos = base + np.arange(n_k)
            cols[pos] = lo + e0 + np.arange(n_k)
            s_t[e0:e0 + n_k] = pos // TILE_E
            s_r[e0:e0 + n_k] = pos % TILE_E

        d = dijk_bf_T[:, cols]                        # [n_in, E_PC]
        d0 = np.ascontiguousarray(d[0:128])
        d1 = np.ascontiguousarray(d[128:256])
        d2 = np.ascontiguousarray(d[256:])
        xgT = np.ascontiguousarray(xgT_all[:, cols])  # [128, E_PC]

        S = np.zeros((p.T, TILE_E, CHUNK_A), dtype=BF16)
        s_c = es - chunk * CHUNK_A
        S[s_t, s_r, s_c] = 1.0

        xT = np.zeros((128, p.NA_TAIL), dtype=np.float32)
        xT[:, : p.NA] = x[c * p.NA : (c + 1) * p.NA].T

        per_core.append(dict(d0=d0, d1=d1, d2=d2, xgT=xgT, s_pages=S, xT=xT))
    return per_core


def build_program(p):
    nc = bacc.Bacc(None, target_bir_lowering=False)

    d0 = nc.declare_dram_parameter("d0", [128, p.E_PC], dt.bfloat16, isOutput=False)
    d1 = nc.declare_dram_parameter("d1", [128, p.E_PC], dt.bfloat16, isOutput=False)
    d2 = nc.declare_dram_parameter("d2", [p.KC[2], p.E_PC], dt.bfloat16, isOutput=False)
    xgT = nc.declare_dram_parameter("xgT", [128, p.E_PC], dt.bfloat16, isOutput=False)
    s_pages = nc.declare_dram_parameter(
        "s_pages", [p.T, TILE_E, CHUNK_A], dt.bfloat16, isOutput=False)
    xT = nc.declare_dram_parameter("xT", [128, p.NA_TAIL], dt.float32, isOutput=False)
    w1b = nc.declare_dram_parameter("w1b", [p.n_in, 128], dt.bfloat16, isOutput=False)
    w2b = nc.declare_dram_parameter("w2b", [128, 128], dt.bfloat16, isOutput=False)
    winb = nc.declare_dram_parameter("winb", [128, 128], dt.bfloat16, isOutput=False)
    woutb = nc.declare_dram_parameter("woutb", [128, 128], dt.bfloat16, isOutput=False)
    wdb = nc.declare_dram_parameter("wdb", [128, 128], dt.bfloat16, isOutput=False)
    identb = nc.declare_dram_parameter("identb", [128, 128], dt.bfloat16, isOutput=False)

    y_out = nc.declare_dram_parameter("y_out", [128, p.NA_TAIL], dt.float32, isOutput=True)
    v_out = nc.declare_dram_parameter("v_out", [128, p.NA_TAIL], dt.float32, isOutput=True)

    dsrc = [d0, d1, d2]

    with tile.TileContext(nc) as tc:
        # keep both Exp and Ln tables resident for the whole program
        nc.scalar.add_instruction(
            mybir.InstLoadActFuncSet(
                name=nc.get_next_instruction_name(), ins=[], outs=[],
                act_func_set_id=ACT_SET_LN_EXP,
            )
        )
        with (
            tc.tile_pool(name="const", bufs=1) as constp,
            tc.tile_pool(name="dload", bufs=2) as dload,
            tc.tile_pool(name="stage", bufs=2) as stage,
            tc.tile_pool(name="work", bufs=2) as work,
            tc.tile_pool(name="tail", bufs=2) as tailp,
            tc.tile_pool(name="ps_t1", bufs=2, space="PSUM") as ps_t1,
            tc.tile_pool(name="ps_w", bufs=2, space="PSUM") as ps_w,
            tc.tile_pool(name="ps_f", bufs=1, space="PSUM") as ps_f,
            tc.tile_pool(name="ps_tr", bufs=1, space="PSUM") as ps_tr,
            tc.tile_pool(name="ps_cv", bufs=2, space="PSUM") as ps_cv,
        ):
            # ---- constants ----
            idn = constp.tile([128, 128], dt.bfloat16)
            nc.sync.dma_start(out=idn[:], in_=identb[:, :])
            half_c = constp.tile([128, 1], dt.float32)
            nc.gpsimd.memset(half_c[:], 0.5)
            w1sb = []
            for kc in range(p.NKC):
                kn = p.KC[kc]
                t = constp.tile([128, 128], dt.bfloat16, name=f"w1sb{kc}")
                nc.sync.dma_start(out=t[:kn, :], in_=w1b[kc * 128: kc * 128 + kn, :])
                w1sb.append(t)
            w2sb = constp.tile([128, 128], dt.bfloat16)
            nc.sync.dma_start(out=w2sb[:], in_=w2b[:, :])
            winsb = constp.tile([128, 128], dt.bfloat16)
            nc.sync.dma_start(out=winsb[:], in_=winb[:, :])
            woutsb = constp.tile([128, 128], dt.bfloat16)
            nc.sync.dma_start(out=woutsb[:], in_=woutb[:, :])
            wdsb = constp.tile([128, 128], dt.bfloat16)
            nc.sync.dma_start(out=wdsb[:], in_=wdb[:, :])
            xT_sb = constp.tile([128, p.NA_TAIL], dt.float32)
            nc.sync.dma_start(out=xT_sb[:], in_=xT[:, :])
            convT = constp.tile([128, p.NA_TAIL], dt.bfloat16)

            conv_tiles = {}

            # ---- edge pipeline ----
            # dijk chunks load 2 groups (1 MiB) per call on the SP HWDGE
            # ring; xgT and S pages load per group on the ACT HWDGE ring so
            # the two rings' latency-serialized issue streams overlap.
            dg2 = None
            for g in range(p.NGRP):
                e0 = g * GRP
                if g % 2 == 0:
                    span = min(2 * GRP, p.E_PC - e0)
                    dg2 = []
                    for kc in range(p.NKC):
                        kn = p.KC[kc]
                        tdg = dload.tile([kn, 2 * GRP], dt.bfloat16,
                                         tag=f"dg{kc}", name=f"dg{kc}")
                        nc.sync.dma_start(
                            out=tdg[:, :span], in_=dsrc[kc][:, e0:e0 + span])
                        dg2.append(tdg)
                doff = (g % 2) * GRP
                dg = [t[:, doff:doff + GRP] for t in dg2]
                xgg = dload.tile([128, GRP], dt.bfloat16, tag="xgg")
                nc.scalar.dma_start(out=xgg[:], in_=xgT[:, e0:e0 + GRP])
                sg = dload.tile([128, 16, CHUNK_A], dt.bfloat16, tag="sg")
                nc.scalar.dma_start(
                    out=sg[:],
                    in_=s_pages[g * 16:(g + 1) * 16, :, :].rearrange(
                        "t pp c -> pp t c", pp=128),
                )

                # phase 1: mm1 + Exp per block; Ln batched over the group
                e1g = stage.tile([128, 4, BLK], dt.float32, tag="e1g")
                for b in range(4):
                    t1 = ps_t1.tile([128, BLK], dt.float32, tag="t1")
                    for kc in range(p.NKC):
                        kn = p.KC[kc]
                        nc.tensor.matmul(
                            t1[:], w1sb[kc][:kn, :],
                            dg[kc][:, b * BLK:(b + 1) * BLK],
                            start=(kc == 0), stop=(kc == p.NKC - 1),
                        )
                    nc.scalar.activation(e1g[:, b, :], t1[:], AF.Exp)
                t1sg = stage.tile([128, 4, BLK], dt.bfloat16, tag="t1sg")
                nc.scalar.activation(
                    t1sg[:], e1g[:], AF.Ln, bias=half_c[:], scale=half_c[:])

                # phase 2: mm2 + Exp per block; Ln batched
                ewg = stage.tile([128, 4, BLK], dt.float32, tag="ewg")
                for b in range(4):
                    wps = ps_w.tile([128, BLK], dt.float32, tag="wps")
                    nc.tensor.matmul(
                        wps[:], w2sb[:], t1sg[:, b, :], start=True, stop=True)
                    nc.scalar.activation(ewg[:, b, :], wps[:], AF.Exp)
                wtg = stage.tile([128, 4, BLK], dt.float32, tag="wtg")
                nc.scalar.activation(
                    wtg[:], ewg[:], AF.Ln, bias=half_c[:], scale=half_c[:])

                # phase 3: mm_f, wf, transpose, mm3 per block
                for b in range(4):
                    fps = ps_f.tile([128, BLK], dt.float32, tag="fps")
                    nc.tensor.matmul(
                        fps[:], winsb[:], xgg[:, b * BLK:(b + 1) * BLK],
                        start=True, stop=True)
                    wfT = work.tile([128, BLK], dt.bfloat16, tag="wfT")
                    nc.vector.tensor_tensor(
                        wfT[:], wtg[:, b, :], fps[:], mybir.AluOpType.mult)
                    wfP = ps_tr.tile([128, BLK], dt.bfloat16, tag="wfP")
                    for i in range(4):
                        nc.tensor.transpose(
                            wfP[:, i * 128:(i + 1) * 128],
                            wfT[:, i * 128:(i + 1) * 128], idn[:])
                    wf = work.tile([128, BLK], dt.bfloat16, tag="wf")
                    nc.vector.tensor_copy(wf[:], wfP[:])

                    for i in range(4):
                        t = g * 16 + b * 4 + i           # global tile id
                        k = p.tile_chunk[t]
                        if p.first_tile[k] == t:
                            cv = ps_cv.tile([128, CHUNK_A], dt.float32,
                                            tag="cv", name="cv")
                            nc.vector.memset(cv[:], 0.0)
                            conv_tiles[k] = cv
                        cv = conv_tiles[k]
                        nc.tensor.matmul(
                            cv[:], wf[:, i * 128:(i + 1) * 128],
                            sg[:, b * 4 + i, :],
                            start=False, stop=(p.last_tile[k] == t),
                            skip_group_check=True,
                        )
                        if p.last_tile[k] == t:
                            nc.vector.tensor_copy(
                                convT[:, k * CHUNK_A:(k + 1) * CHUNK_A], cv[:])
                            del conv_tiles[k]

            # ---- tail: z3^T = Wout.T @ conv^T, ssp, v^T, y^T ----
            for s in range(p.NSLAB):
                a0 = s * 512
                z3 = ps_t1.tile([128, 512], dt.float32, tag="t1", name="z3")
                nc.tensor.matmul(
                    z3[:], woutsb[:], convT[:, a0:a0 + 512], start=True, stop=True)
                e3 = tailp.tile([128, 512], dt.float32, tag="e3")
                nc.scalar.activation(e3[:], z3[:], AF.Exp)
                hT = tailp.tile([128, 512], dt.bfloat16, tag="hT")
                nc.scalar.activation(
                    hT[:], e3[:], AF.Ln, bias=half_c[:], scale=half_c[:])
                vps = ps_w.tile([128, 512], dt.float32, tag="wps", name="vps")
                nc.tensor.matmul(vps[:], wdsb[:], hT[:], start=True, stop=True)
                v_sb = tailp.tile([128, 512], dt.float32, tag="v_sb")
                nc.vector.tensor_copy(v_sb[:], vps[:])
                nc.sync.dma_start(out=v_out[:, a0:a0 + 512], in_=v_sb[:])
                y_sb = tailp.tile([128, 512], dt.float32, tag="y_sb")
                nc.vector.tensor_tensor(
                    y_sb[:], vps[:], xT_sb[:, a0:a0 + 512], mybir.AluOpType.add)
                nc.sync.dma_start(out=y_out[:, a0:a0 + 512], in_=y_sb[:])

    nc.finalize()
    return nc


_PROG_CACHE = {}


def kernel(x, dijk, W1, b1, W2, b2, Win, Wout, bout, Wd, bd, idx_j, seg_i, seg_j):
    x = np.ascontiguousarray(np.asarray(x, dtype=np.float32))
    dijk = np.ascontiguousarray(np.asarray(dijk, dtype=np.float32))
    for b in (b1, b2, bout, bd):
        assert np.abs(np.asarray(b)).max() == 0.0, "nonzero biases unsupported"

    n_atoms, n_basis = x.shape
    n_edges, n_in = dijk.shape
    assert n_basis == 128 and np.asarray(W2).shape == (128, 128)

    p = Plan(n_atoms, n_edges, n_in, seg_i)

    # global host-side layout transforms (shared across cores)
    dijk_bf_T = np.zeros((n_in, n_edges + 1), dtype=BF16)
    dijk_bf_T[:, :n_edges] = _to_bf16(dijk).T
    x_bf = _to_bf16(x)
    idx = np.asarray(idx_j).astype(np.int64)
    xgT_all = np.zeros((128, n_edges + 1), dtype=BF16)
    xgT_all[:, :n_edges] = x_bf[idx].T

    per_core = shard_inputs(p, x, dijk_bf_T, xgT_all, seg_i)
    del dijk_bf_T, xgT_all

    key = (n_atoms, n_edges, n_in, tuple(p.TPW))
    if key not in _PROG_CACHE:
        _PROG_CACHE[key] = build_program(p)
    nc = _PROG_CACHE[key]

    common = dict(
        w1b=_to_bf16(np.asarray(W1, dtype=np.float32)),
        w2b=_to_bf16(np.asarray(W2, dtype=np.float32)),
        winb=_to_bf16(np.asarray(Win, dtype=np.float32)),
        woutb=_to_bf16(np.asarray(Wout, dtype=np.float32)),
        wdb=_to_bf16(np.asarray(Wd, dtype=np.float32)),
        identb=_to_bf16(np.eye(128, dtype=np.float32)),
    )
    in_maps = [{**common, **pc} for pc in per_core]
    res = run_bass_kernel_spmd(nc, in_maps, list(range(N_CORES)))
    global LAST_RESULTS
    LAST_RESULTS = res

    y = np.empty((n_atoms, 128), dtype=np.float32)
    v = np.empty((n_atoms, 128), dtype=np.float32)
    for c in range(N_CORES):
        y[c * p.NA:(c + 1) * p.NA] = res.results[c]["y_out"][:, : p.NA].T
        v[c * p.NA:(c + 1) * p.NA] = res.results[c]["v_out"][:, : p.NA].T
    return (y, v)


# revision 16
# speedup vs baseline: 1.0108x; 1.0108x over previous
"""CFNet interaction block on 8 trn2 NeuronCores (SPMD bass/tile kernel), v2.

Per core c of 8 (SPMD, one program, per-core data): core c owns atoms
[c*NA, (c+1)*NA) and the edges whose sorted seg_i lands there.

Host prep = pure layout (no reference FLOPs):
  - dijk cast fp32->bf16 and pre-TRANSPOSED into k-chunks [128|128|44, E_PC]
    (kills the device-side PE transposes and halves dijk HBM traffic),
  - x rows pre-gathered by idx_j, transposed: xg^T [128x, E_PC] bf16
    (kills the v1 per-edge dma_gather: ~9 ns of Q7 per edge),
  - one-hot S pages [T, 128, 128] bf16: edge row -> column (atom mod 128).

Static SPMD schedule: local atoms are split into 128-atom chunks; chunk k
gets a FIXED tile allotment TPW[k] (mean + 6 sigma), so every core's tile t
maps to the same chunk and the same psum window -- per-core variation is
absorbed by padding (~7% edge inflation).  Pad edges carry all-zero S rows.

Device pipeline per 512-edge block, [feature, edge] layout (weights are the
matmul stationaries):
  mm1  t1 = W1c.T @ dT (3 k-chunks)          psum [f1, e]
  ssp1 Exp (psum->sbuf, per block), Ln(0.5x+0.5) batched [128, 2048] -> bf16
  mm2  w^T = W2.T @ t1s                      psum [f2, e]
  sspw Exp per block, Ln batched             -> wt^T fp32
  mm_f f^T = Win.T @ xg^T                    psum [f, e]
  wf^T = wt^T * f^T (DVE)                    -> bf16
  PE-transpose wf^T -> wf [e, f] (psum bf16 -> sbuf)
  mm3 per 128-edge tile: conv^T[chunk] += wf_tile.T @ S_tile
       into a [128, 128] psum window per chunk; ~49 flushes to sbuf.
A single manual InstLoadActFuncSet(natural_log_exp_and_others) keeps Exp+Ln
resident: no ACT_TABLE_LOAD thrash (was 2.6 ms in v1).

Tail from sbuf-resident conv^T: z3^T = Wout.T @ conv^T, ssp, v^T = Wd.T @
h^T, y^T = v^T + x^T.  Outputs leave TRANSPOSED [128, NA_PAD]; the host
transposes back.  ssp(x) = Ln(0.5 + 0.5*Exp(x)) exactly.
"""

import math
import sys

import numpy as np
import ml_dtypes

sys.path.insert(0, "/opt/trn_rl_repo")

import concourse.bacc as bacc
import concourse.mybir as mybir
from concourse import tile
from concourse.bass_utils import run_bass_kernel_spmd

dt = mybir.dt
AF = mybir.ActivationFunctionType
BF16 = ml_dtypes.bfloat16

N_CORES = 8
TILE_E = 128            # edges per S tile / conv matmul
BLK = 512               # edges per pipeline block
GRP = 2048              # edges per DMA group (4 blocks, 16 tiles)
CHUNK_A = 128           # atoms per conv psum window
ACT_SET_LN_EXP = 6      # natural_log_exp_and_others in act_info.json


def _ceil(a, b):
    return -(-a // b)


def _to_bf16(a):
    """fp32 -> bf16 with round-to-nearest-even, fast numpy path."""
    a = np.ascontiguousarray(a, dtype=np.float32)
    v = a.view(np.uint32)
    r = ((v + np.uint32(0x7FFF) + ((v >> np.uint32(16)) & np.uint32(1)))
         >> np.uint32(16)).astype(np.uint16)
    return r.view(BF16).reshape(a.shape)


class Plan:
    """Structure constants; identical for every core.  The per-chunk tile
    allotment TPW is fitted to the ACTUAL seg_i data (max over cores), so
    padding is just tile rounding (~3%) and fits by construction."""

    def __init__(self, n_atoms, n_edges, n_in, seg_i):
        assert n_atoms % N_CORES == 0
        self.n_atoms, self.n_edges, self.n_in = n_atoms, n_edges, n_in
        self.NA = n_atoms // N_CORES
        self.NCHUNK_REAL = _ceil(self.NA, CHUNK_A)
        seg_i = np.asarray(seg_i).astype(np.int64)
        bounds = np.searchsorted(seg_i, np.arange(N_CORES + 1) * self.NA)
        mx = np.zeros(self.NCHUNK_REAL, dtype=np.int64)
        for c in range(N_CORES):
            es = seg_i[bounds[c]:bounds[c + 1]] - c * self.NA
            cnt = np.bincount(es // CHUNK_A, minlength=self.NCHUNK_REAL)
            mx = np.maximum(mx, cnt)
        tpw = [int(_ceil(int(m), TILE_E)) for m in mx]
        t_raw = sum(tpw)
        self.T = int(_ceil(t_raw, GRP // TILE_E) * (GRP // TILE_E))
        self.TPW = tpw
        self.E_PC = self.T * TILE_E
        self.NBLK = self.E_PC // BLK
        self.NGRP = self.E_PC // GRP
        self.KC = [min(128, n_in - 128 * i) for i in range(_ceil(n_in, 128))]
        self.NKC = len(self.KC)
        # tile -> chunk map; trailing pad tiles attach to the last chunk
        self.first_tile = []
        self.tile_chunk = []
        for k, n in enumerate(tpw):
            self.first_tile.append(len(self.tile_chunk))
            self.tile_chunk += [k] * n
        self.tile_chunk += [self.NCHUNK_REAL - 1] * (self.T - t_raw)
        self.last_tile = [0] * self.NCHUNK_REAL
        for t, k in enumerate(self.tile_chunk):
            self.last_tile[k] = t
        self.NA_PAD = self.NCHUNK_REAL * CHUNK_A
        self.NSLAB = _ceil(self.NA_PAD, 512)         # tail slabs of 512 atoms
        self.NA_TAIL = self.NSLAB * 512


def shard_inputs(p, x, dijk_bf_T, xgT_all, seg_i):
    """Per-core layout prep. dijk_bf_T/xgT_all carry a zero pad column at
    index n_edges."""
    seg_i = np.asarray(seg_i).astype(np.int64)
    bounds = np.searchsorted(seg_i, np.arange(N_CORES + 1) * p.NA)
    ZCOL = p.n_edges                                  # the zero column

    per_core = []
    for c in range(N_CORES):
        lo, hi = int(bounds[c]), int(bounds[c + 1])
        es = seg_i[lo:hi] - c * p.NA                  # local atoms, sorted
        chunk = es // CHUNK_A

        # per-chunk edge placement into the static tile schedule
        cols = np.full(p.E_PC, ZCOL, dtype=np.int64)  # global edge id or pad
        s_t = np.empty(hi - lo, dtype=np.int64)       # tile of each edge
        s_r = np.empty(hi - lo, dtype=np.int64)       # row within tile
        cnt = np.bincount(chunk, minlength=p.NCHUNK_REAL)
        for k in range(p.NCHUNK_REAL):
            n_k = int(cnt[k])
            if n_k == 0:
                continue
            assert n_k <= p.TPW[k] * TILE_E, (c, k, n_k, p.TPW[k] * TILE_E)
            e0 = int(np.searchsorted(chunk, k))
            base = p.first_tile[k] * TILE_E
            pos = base + np.arange(n_k)
            cols[pos] = lo + e0 + np.arange(n_k)
            s_t[e0:e0 + n_k] = pos // TILE_E
            s_r[e0:e0 + n_k] = pos % TILE_E

        d = dijk_bf_T[:, cols]                        # [n_in, E_PC]
        d0 = np.ascontiguousarray(d[0:128])
        d1 = np.ascontiguousarray(d[128:256])
        d2 = np.ascontiguousarray(d[256:])
        xgT = np.ascontiguousarray(xgT_all[:, cols])  # [128, E_PC]

        S = np.zeros((p.T, TILE_E, CHUNK_A), dtype=BF16)
        s_c = es - chunk * CHUNK_A
        S[s_t, s_r, s_c] = 1.0

        xT = np.zeros((128, p.NA_TAIL), dtype=np.float32)
        xT[:, : p.NA] = x[c * p.NA : (c + 1) * p.NA].T

        per_core.append(dict(d0=d0, d1=d1, d2=d2, xgT=xgT, s_pages=S, xT=xT))
    return per_core


def build_program(p):
    nc = bacc.Bacc(None, target_bir_lowering=False)

    d0 = nc.declare_dram_parameter("d0", [128, p.E_PC], dt.bfloat16, isOutput=False)
    d1 = nc.declare_dram_parameter("d1", [128, p.E_PC], dt.bfloat16, isOutput=False)
    d2 = nc.declare_dram_parameter("d2", [p.KC[2], p.E_PC], dt.bfloat16, isOutput=False)
    xgT = nc.declare_dram_parameter("xgT", [128, p.E_PC], dt.bfloat16, isOutput=False)
    s_pages = nc.declare_dram_parameter(
        "s_pages", [p.T, TILE_E, CHUNK_A], dt.bfloat16, isOutput=False)
    xT = nc.declare_dram_parameter("xT", [128, p.NA_TAIL], dt.float32, isOutput=False)
    w1b = nc.declare_dram_parameter("w1b", [p.n_in, 128], dt.bfloat16, isOutput=False)
    w2b = nc.declare_dram_parameter("w2b", [128, 128], dt.bfloat16, isOutput=False)
    winb = nc.declare_dram_parameter("winb", [128, 128], dt.bfloat16, isOutput=False)
    woutb = nc.declare_dram_parameter("woutb", [128, 128], dt.bfloat16, isOutput=False)
    wdb = nc.declare_dram_parameter("wdb", [128, 128], dt.bfloat16, isOutput=False)
    identb = nc.declare_dram_parameter("identb", [128, 128], dt.bfloat16, isOutput=False)

    y_out = nc.declare_dram_parameter("y_out", [128, p.NA_TAIL], dt.float32, isOutput=True)
    v_out = nc.declare_dram_parameter("v_out", [128, p.NA_TAIL], dt.float32, isOutput=True)

    dsrc = [d0, d1, d2]

    with tile.TileContext(nc) as tc:
        # keep both Exp and Ln tables resident for the whole program
        nc.scalar.add_instruction(
            mybir.InstLoadActFuncSet(
                name=nc.get_next_instruction_name(), ins=[], outs=[],
                act_func_set_id=ACT_SET_LN_EXP,
            )
        )
        with (
            tc.tile_pool(name="const", bufs=1) as constp,
            tc.tile_pool(name="dload", bufs=2) as dload,
            tc.tile_pool(name="stage", bufs=2) as stage,
            tc.tile_pool(name="work", bufs=2) as work,
            tc.tile_pool(name="tail", bufs=2) as tailp,
            tc.tile_pool(name="ps_t1", bufs=2, space="PSUM") as ps_t1,
            tc.tile_pool(name="ps_w", bufs=2, space="PSUM") as ps_w,
            tc.tile_pool(name="ps_f", bufs=1, space="PSUM") as ps_f,
            tc.tile_pool(name="ps_tr", bufs=2, space="PSUM") as ps_tr,
            tc.tile_pool(name="ps_cv", bufs=1, space="PSUM") as ps_cv,
        ):
            # ---- constants ----
            idn = constp.tile([128, 128], dt.bfloat16)
            nc.sync.dma_start(out=idn[:], in_=identb[:, :])
            half_c = constp.tile([128, 1], dt.float32)
            nc.gpsimd.memset(half_c[:], 0.5)
            w1sb = []
            for kc in range(p.NKC):
                kn = p.KC[kc]
                t = constp.tile([128, 128], dt.bfloat16, name=f"w1sb{kc}")
                nc.sync.dma_start(out=t[:kn, :], in_=w1b[kc * 128: kc * 128 + kn, :])
                w1sb.append(t)
            w2sb = constp.tile([128, 128], dt.bfloat16)
            nc.sync.dma_start(out=w2sb[:], in_=w2b[:, :])
            winsb = constp.tile([128, 128], dt.bfloat16)
            nc.sync.dma_start(out=winsb[:], in_=winb[:, :])
            woutsb = constp.tile([128, 128], dt.bfloat16)
            nc.sync.dma_start(out=woutsb[:], in_=woutb[:, :])
            wdsb = constp.tile([128, 128], dt.bfloat16)
            nc.sync.dma_start(out=wdsb[:], in_=wdb[:, :])
            xT_sb = constp.tile([128, p.NA_TAIL], dt.float32)
            nc.sync.dma_start(out=xT_sb[:], in_=xT[:, :])
            convT = constp.tile([128, p.NA_TAIL], dt.bfloat16)

            conv_tiles = {}
            pending = []

            def emit_phase3b(item):
                gq, wfT_list, sgq = item
                for b in range(4):
                    wfT = wfT_list[b]
                    wfP = ps_tr.tile([128, BLK], dt.bfloat16, tag="wfP",
                                     name="wfP")
                    for i in range(4):
                        nc.tensor.transpose(
                            wfP[:, i * 128:(i + 1) * 128],
                            wfT[:, i * 128:(i + 1) * 128], idn[:])
                    wf = work.tile([128, BLK], dt.bfloat16, tag="wf",
                                   name="wf")
                    nc.vector.tensor_copy(wf[:], wfP[:])
                    for i in range(4):
                        t = gq * 16 + b * 4 + i          # global tile id
                        k = p.tile_chunk[t]
                        if p.first_tile[k] == t:
                            cv = ps_cv.tile([128, CHUNK_A], dt.float32,
                                            tag="cv", name="cv")
                            nc.vector.memset(cv[:], 0.0)
                            conv_tiles[k] = cv
                        cv = conv_tiles[k]
                        nc.tensor.matmul(
                            cv[:], wf[:, i * 128:(i + 1) * 128],
                            sgq[:, b * 4 + i, :],
                            start=False, stop=(p.last_tile[k] == t),
                            skip_group_check=True,
                        )
                        if p.last_tile[k] == t:
                            nc.vector.tensor_copy(
                                convT[:, k * CHUNK_A:(k + 1) * CHUNK_A],
                                cv[:])
                            del conv_tiles[k]

            # ---- edge pipeline ----
            # dijk chunks load 2 groups (1 MiB) per call on the SP HWDGE
            # ring; xgT and S pages load per group on the ACT HWDGE ring so
            # the two rings' latency-serialized issue streams overlap.
            dg2 = None
            for g in range(p.NGRP):
                e0 = g * GRP
                if g % 2 == 0:
                    span = min(2 * GRP, p.E_PC - e0)
                    dg2 = []
                    for kc in range(p.NKC):
                        kn = p.KC[kc]
                        tdg = dload.tile([kn, 2 * GRP], dt.bfloat16,
                                         tag=f"dg{kc}", name=f"dg{kc}")
                        nc.sync.dma_start(
                            out=tdg[:, :span], in_=dsrc[kc][:, e0:e0 + span])
                        dg2.append(tdg)
                doff = (g % 2) * GRP
                dg = [t[:, doff:doff + GRP] for t in dg2]
                xgg = dload.tile([128, GRP], dt.bfloat16, tag="xgg")
                nc.scalar.dma_start(out=xgg[:], in_=xgT[:, e0:e0 + GRP])
                sg = dload.tile([128, 16, CHUNK_A], dt.bfloat16, tag="sg",
                                bufs=3)
                nc.scalar.dma_start(
                    out=sg[:],
                    in_=s_pages[g * 16:(g + 1) * 16, :, :].rearrange(
                        "t pp c -> pp t c", pp=128),
                )

                # phase 1: mm1 + Exp per block; Ln batched over the group
                e1g = stage.tile([128, 4, BLK], dt.float32, tag="e1g")
                for b in range(4):
                    t1 = ps_t1.tile([128, BLK], dt.float32, tag="t1")
                    for kc in range(p.NKC):
                        kn = p.KC[kc]
                        nc.tensor.matmul(
                            t1[:], w1sb[kc][:kn, :],
                            dg[kc][:, b * BLK:(b + 1) * BLK],
                            start=(kc == 0), stop=(kc == p.NKC - 1),
                        )
                    nc.scalar.activation(e1g[:, b, :], t1[:], AF.Exp)
                t1sg = stage.tile([128, 4, BLK], dt.bfloat16, tag="t1sg")
                nc.scalar.activation(
                    t1sg[:], e1g[:], AF.Ln, bias=half_c[:], scale=half_c[:])

                # phase 2: mm2 + Exp per block; Ln batched
                ewg = stage.tile([128, 4, BLK], dt.float32, tag="ewg")
                for b in range(4):
                    wps = ps_w.tile([128, BLK], dt.float32, tag="wps")
                    nc.tensor.matmul(
                        wps[:], w2sb[:], t1sg[:, b, :], start=True, stop=True)
                    nc.scalar.activation(ewg[:, b, :], wps[:], AF.Exp)
                wtg = stage.tile([128, 4, BLK], dt.float32, tag="wtg")
                nc.scalar.activation(
                    wtg[:], ewg[:], AF.Ln, bias=half_c[:], scale=half_c[:])

                # phase 3a: mm_f + wf multiply (psum freed immediately);
                # wfT tiles are held one group so phase 3b can lag.
                wfT_g = []
                for b in range(4):
                    fps = ps_f.tile([128, BLK], dt.float32, tag="fps")
                    nc.tensor.matmul(
                        fps[:], winsb[:], xgg[:, b * BLK:(b + 1) * BLK],
                        start=True, stop=True)
                    wfT = work.tile([128, BLK], dt.bfloat16, tag="wfT",
                                    bufs=8)
                    nc.vector.tensor_tensor(
                        wfT[:], wtg[:, b, :], fps[:], mybir.AluOpType.mult)
                    wfT_g.append(wfT)
                pending.append((g, wfT_g, sg))

                # phase 3b for the PREVIOUS group: transposes + mm3 never
                # sit at the PE queue head waiting on this group's ACT.
                if len(pending) > 1:
                    emit_phase3b(pending.pop(0))

            emit_phase3b(pending.pop(0))

            # ---- tail: z3^T = Wout.T @ conv^T, ssp, v^T, y^T ----
            for s in range(p.NSLAB):
                a0 = s * 512
                z3 = ps_t1.tile([128, 512], dt.float32, tag="t1", name="z3")
                nc.tensor.matmul(
                    z3[:], woutsb[:], convT[:, a0:a0 + 512], start=True, stop=True)
                e3 = tailp.tile([128, 512], dt.float32, tag="e3")
                nc.scalar.activation(e3[:], z3[:], AF.Exp)
                hT = tailp.tile([128, 512], dt.bfloat16, tag="hT")
                nc.scalar.activation(
                    hT[:], e3[:], AF.Ln, bias=half_c[:], scale=half_c[:])
                vps = ps_w.tile([128, 512], dt.float32, tag="wps", name="vps")
                nc.tensor.matmul(vps[:], wdsb[:], hT[:], start=True, stop=True)
                v_sb = tailp.tile([128, 512], dt.float32, tag="v_sb")
                nc.vector.tensor_copy(v_sb[:], vps[:])
                nc.sync.dma_start(out=v_out[:, a0:a0 + 512], in_=v_sb[:])
                y_sb = tailp.tile([128, 512], dt.float32, tag="y_sb")
                nc.vector.tensor_tensor(
                    y_sb[:], vps[:], xT_sb[:, a0:a0 + 512], mybir.AluOpType.add)
                nc.sync.dma_start(out=y_out[:, a0:a0 + 512], in_=y_sb[:])

    nc.finalize()
    return nc


_PROG_CACHE = {}


def kernel(x, dijk, W1, b1, W2, b2, Win, Wout, bout, Wd, bd, idx_j, seg_i, seg_j):
    x = np.ascontiguousarray(np.asarray(x, dtype=np.float32))
    dijk = np.ascontiguousarray(np.asarray(dijk, dtype=np.float32))
    for b in (b1, b2, bout, bd):
        assert np.abs(np.asarray(b)).max() == 0.0, "nonzero biases unsupported"

    n_atoms, n_basis = x.shape
    n_edges, n_in = dijk.shape
    assert n_basis == 128 and np.asarray(W2).shape == (128, 128)

    p = Plan(n_atoms, n_edges, n_in, seg_i)

    # global host-side layout transforms (shared across cores)
    dijk_bf_T = np.zeros((n_in, n_edges + 1), dtype=BF16)
    dijk_bf_T[:, :n_edges] = _to_bf16(dijk).T
    x_bf = _to_bf16(x)
    idx = np.asarray(idx_j).astype(np.int64)
    xgT_all = np.zeros((128, n_edges + 1), dtype=BF16)
    xgT_all[:, :n_edges] = x_bf[idx].T

    per_core = shard_inputs(p, x, dijk_bf_T, xgT_all, seg_i)
    del dijk_bf_T, xgT_all

    key = (n_atoms, n_edges, n_in, tuple(p.TPW))
    if key not in _PROG_CACHE:
        _PROG_CACHE[key] = build_program(p)
    nc = _PROG_CACHE[key]

    common = dict(
        w1b=_to_bf16(np.asarray(W1, dtype=np.float32)),
        w2b=_to_bf16(np.asarray(W2, dtype=np.float32)),
        winb=_to_bf16(np.asarray(Win, dtype=np.float32)),
        woutb=_to_bf16(np.asarray(Wout, dtype=np.float32)),
        wdb=_to_bf16(np.asarray(Wd, dtype=np.float32)),
        identb=_to_bf16(np.eye(128, dtype=np.float32)),
    )
    in_maps = [{**common, **pc} for pc in per_core]
    res = run_bass_kernel_spmd(nc, in_maps, list(range(N_CORES)))
    global LAST_RESULTS
    LAST_RESULTS = res

    y = np.empty((n_atoms, 128), dtype=np.float32)
    v = np.empty((n_atoms, 128), dtype=np.float32)
    for c in range(N_CORES):
        y[c * p.NA:(c + 1) * p.NA] = res.results[c]["y_out"][:, : p.NA].T
        v[c * p.NA:(c + 1) * p.NA] = res.results[c]["v_out"][:, : p.NA].T
    return (y, v)


# revision 18
# speedup vs baseline: 1.4818x; 1.4659x over previous
"""CFNet interaction block on 8 trn2 NeuronCores (SPMD bass/tile kernel), v2.

Per core c of 8 (SPMD, one program, per-core data): core c owns atoms
[c*NA, (c+1)*NA) and the edges whose sorted seg_i lands there.

Host prep = pure layout (no reference FLOPs):
  - dijk cast fp32->bf16 and pre-TRANSPOSED into k-chunks [128|128|44, E_PC]
    (kills the device-side PE transposes and halves dijk HBM traffic),
  - x rows pre-gathered by idx_j, transposed: xg^T [128x, E_PC] bf16
    (kills the v1 per-edge dma_gather: ~9 ns of Q7 per edge),
  - one-hot S pages [T, 128, 128] bf16: edge row -> column (atom mod 128).

Static SPMD schedule: local atoms are split into 128-atom chunks; chunk k
gets a FIXED tile allotment TPW[k] (mean + 6 sigma), so every core's tile t
maps to the same chunk and the same psum window -- per-core variation is
absorbed by padding (~7% edge inflation).  Pad edges carry all-zero S rows.

Device pipeline per 512-edge block, [feature, edge] layout (weights are the
matmul stationaries):
  mm1  t1 = W1c.T @ dT (3 k-chunks)          psum [f1, e]
  ssp1 Exp (psum->sbuf, per block), Ln(0.5x+0.5) batched [128, 2048] -> bf16
  mm2  w^T = W2.T @ t1s                      psum [f2, e]
  sspw Exp per block, Ln batched             -> wt^T fp32
  mm_f f^T = Win.T @ xg^T                    psum [f, e]
  wf^T = wt^T * f^T (DVE)                    -> bf16
  PE-transpose wf^T -> wf [e, f] (psum bf16 -> sbuf)
  mm3 per 128-edge tile: conv^T[chunk] += wf_tile.T @ S_tile
       into a [128, 128] psum window per chunk; ~49 flushes to sbuf.
A single manual InstLoadActFuncSet(natural_log_exp_and_others) keeps Exp+Ln
resident: no ACT_TABLE_LOAD thrash (was 2.6 ms in v1).

Tail from sbuf-resident conv^T: z3^T = Wout.T @ conv^T, ssp, v^T = Wd.T @
h^T, y^T = v^T + x^T.  Outputs leave TRANSPOSED [128, NA_PAD]; the host
transposes back.  ssp(x) = Ln(0.5 + 0.5*Exp(x)) exactly.
"""

import math
import sys

import numpy as np
import ml_dtypes

sys.path.insert(0, "/opt/trn_rl_repo")

import concourse.bacc as bacc
import concourse.mybir as mybir
from concourse import tile
from concourse.bass_utils import run_bass_kernel_spmd

dt = mybir.dt
AF = mybir.ActivationFunctionType
BF16 = ml_dtypes.bfloat16

N_CORES = 8
TILE_E = 128            # edges per S tile / conv matmul
BLK = 512               # edges per pipeline block
GRP = 2048              # edges per DMA group (4 blocks, 16 tiles)
CHUNK_A = 128           # atoms per conv psum window
ACT_SET_LN_EXP = 6      # natural_log_exp_and_others in act_info.json


def _ceil(a, b):
    return -(-a // b)


def _to_bf16(a):
    """fp32 -> bf16 with round-to-nearest-even, fast numpy path."""
    a = np.ascontiguousarray(a, dtype=np.float32)
    v = a.view(np.uint32)
    r = ((v + np.uint32(0x7FFF) + ((v >> np.uint32(16)) & np.uint32(1)))
         >> np.uint32(16)).astype(np.uint16)
    return r.view(BF16).reshape(a.shape)


class Plan:
    """Structure constants; identical for every core.  The per-chunk tile
    allotment TPW is fitted to the ACTUAL seg_i data (max over cores), so
    padding is just tile rounding (~3%) and fits by construction."""

    def __init__(self, n_atoms, n_edges, n_in, seg_i):
        assert n_atoms % N_CORES == 0
        self.n_atoms, self.n_edges, self.n_in = n_atoms, n_edges, n_in
        self.NA = n_atoms // N_CORES
        self.NCHUNK_REAL = _ceil(self.NA, CHUNK_A)
        seg_i = np.asarray(seg_i).astype(np.int64)
        bounds = np.searchsorted(seg_i, np.arange(N_CORES + 1) * self.NA)
        mx = np.zeros(self.NCHUNK_REAL, dtype=np.int64)
        for c in range(N_CORES):
            es = seg_i[bounds[c]:bounds[c + 1]] - c * self.NA
            cnt = np.bincount(es // CHUNK_A, minlength=self.NCHUNK_REAL)
            mx = np.maximum(mx, cnt)
        tpw = [int(_ceil(int(m), TILE_E)) for m in mx]
        t_raw = sum(tpw)
        self.T = int(_ceil(t_raw, GRP // TILE_E) * (GRP // TILE_E))
        self.TPW = tpw
        self.E_PC = self.T * TILE_E
        self.NBLK = self.E_PC // BLK
        self.NGRP = self.E_PC // GRP
        self.KC = [min(128, n_in - 128 * i) for i in range(_ceil(n_in, 128))]
        self.NKC = len(self.KC)
        # tile -> chunk map; trailing pad tiles attach to the last chunk
        self.first_tile = []
        self.tile_chunk = []
        for k, n in enumerate(tpw):
            self.first_tile.append(len(self.tile_chunk))
            self.tile_chunk += [k] * n
        self.tile_chunk += [self.NCHUNK_REAL - 1] * (self.T - t_raw)
        self.last_tile = [0] * self.NCHUNK_REAL
        for t, k in enumerate(self.tile_chunk):
            self.last_tile[k] = t
        self.NA_PAD = self.NCHUNK_REAL * CHUNK_A
        self.NSLAB = _ceil(self.NA_PAD, 512)         # tail slabs of 512 atoms
        self.NA_TAIL = self.NSLAB * 512


def shard_inputs(p, x, dijk_bf_T, xgT_all, seg_i):
    """Per-core layout prep. dijk_bf_T/xgT_all carry a zero pad column at
    index n_edges."""
    seg_i = np.asarray(seg_i).astype(np.int64)
    bounds = np.searchsorted(seg_i, np.arange(N_CORES + 1) * p.NA)
    ZCOL = p.n_edges                                  # the zero column

    per_core = []
    for c in range(N_CORES):
        lo, hi = int(bounds[c]), int(bounds[c + 1])
        es = seg_i[lo:hi] - c * p.NA                  # local atoms, sorted
        chunk = es // CHUNK_A

        # per-chunk edge placement into the static tile schedule
        cols = np.full(p.E_PC, ZCOL, dtype=np.int64)  # global edge id or pad
        s_t = np.empty(hi - lo, dtype=np.int64)       # tile of each edge
        s_r = np.empty(hi - lo, dtype=np.int64)       # row within tile
        cnt = np.bincount(chunk, minlength=p.NCHUNK_REAL)
        for k in range(p.NCHUNK_REAL):
            n_k = int(cnt[k])
            if n_k == 0:
                continue
            assert n_k <= p.TPW[k] * TILE_E, (c, k, n_k, p.TPW[k] * TILE_E)
            e0 = int(np.searchsorted(chunk, k))
            base = p.first_tile[k] * TILE_E
            pos = base + np.arange(n_k)
            cols[pos] = lo + e0 + np.arange(n_k)
            s_t[e0:e0 + n_k] = pos // TILE_E
            s_r[e0:e0 + n_k] = pos % TILE_E

        d = dijk_bf_T[:, cols]                        # [n_in, E_PC]
        d0 = np.ascontiguousarray(d[0:128])
        d1 = np.ascontiguousarray(d[128:256])
        d2 = np.ascontiguousarray(d[256:])
        xgT = np.ascontiguousarray(xgT_all[:, cols])  # [128, E_PC]

        S = np.zeros((p.T, TILE_E, CHUNK_A), dtype=BF16)
        s_c = es - chunk * CHUNK_A
        S[s_t, s_r, s_c] = 1.0

        xT = np.zeros((128, p.NA_TAIL), dtype=np.float32)
        xT[:, : p.NA] = x[c * p.NA : (c + 1) * p.NA].T

        per_core.append(dict(d0=d0, d1=d1, d2=d2, xgT=xgT, s_pages=S, xT=xT))
    return per_core


def build_program(p):
    nc = bacc.Bacc(None, target_bir_lowering=False)

    d0 = nc.declare_dram_parameter("d0", [128, p.E_PC], dt.bfloat16, isOutput=False)
    d1 = nc.declare_dram_parameter("d1", [128, p.E_PC], dt.bfloat16, isOutput=False)
    d2 = nc.declare_dram_parameter("d2", [p.KC[2], p.E_PC], dt.bfloat16, isOutput=False)
    xgT = nc.declare_dram_parameter("xgT", [128, p.E_PC], dt.bfloat16, isOutput=False)
    s_pages = nc.declare_dram_parameter(
        "s_pages", [p.T, TILE_E, CHUNK_A], dt.bfloat16, isOutput=False)
    xT = nc.declare_dram_parameter("xT", [128, p.NA_TAIL], dt.float32, isOutput=False)
    w1b = nc.declare_dram_parameter("w1b", [p.n_in, 128], dt.bfloat16, isOutput=False)
    w2b = nc.declare_dram_parameter("w2b", [128, 128], dt.bfloat16, isOutput=False)
    winb = nc.declare_dram_parameter("winb", [128, 128], dt.bfloat16, isOutput=False)
    woutb = nc.declare_dram_parameter("woutb", [128, 128], dt.bfloat16, isOutput=False)
    wdb = nc.declare_dram_parameter("wdb", [128, 128], dt.bfloat16, isOutput=False)
    identb = nc.declare_dram_parameter("identb", [128, 128], dt.bfloat16, isOutput=False)

    y_out = nc.declare_dram_parameter("y_out", [128, p.NA_TAIL], dt.float32, isOutput=True)
    v_out = nc.declare_dram_parameter("v_out", [128, p.NA_TAIL], dt.float32, isOutput=True)

    dsrc = [d0, d1, d2]

    with tile.TileContext(nc) as tc:
        # keep both Exp and Ln tables resident for the whole program
        nc.scalar.add_instruction(
            mybir.InstLoadActFuncSet(
                name=nc.get_next_instruction_name(), ins=[], outs=[],
                act_func_set_id=ACT_SET_LN_EXP,
            )
        )
        with (
            tc.tile_pool(name="const", bufs=1) as constp,
            tc.tile_pool(name="dload", bufs=2) as dload,
            tc.tile_pool(name="stage", bufs=2) as stage,
            tc.tile_pool(name="work", bufs=2) as work,
            tc.tile_pool(name="tail", bufs=2) as tailp,
            tc.tile_pool(name="ps_t1", bufs=2, space="PSUM") as ps_t1,
            tc.tile_pool(name="ps_w", bufs=2, space="PSUM") as ps_w,
            tc.tile_pool(name="ps_f", bufs=1, space="PSUM") as ps_f,
            tc.tile_pool(name="ps_tr", bufs=2, space="PSUM") as ps_tr,
            tc.tile_pool(name="ps_cv", bufs=1, space="PSUM") as ps_cv,
        ):
            # ---- constants ----
            idn = constp.tile([128, 128], dt.bfloat16)
            nc.sync.dma_start(out=idn[:], in_=identb[:, :])
            half_c = constp.tile([128, 1], dt.float32)
            nc.gpsimd.memset(half_c[:], 0.5)
            w1sb = []
            for kc in range(p.NKC):
                kn = p.KC[kc]
                t = constp.tile([128, 128], dt.bfloat16, name=f"w1sb{kc}")
                nc.sync.dma_start(out=t[:kn, :], in_=w1b[kc * 128: kc * 128 + kn, :])
                w1sb.append(t)
            w2sb = constp.tile([128, 128], dt.bfloat16)
            nc.sync.dma_start(out=w2sb[:], in_=w2b[:, :])
            winsb = constp.tile([128, 128], dt.bfloat16)
            nc.sync.dma_start(out=winsb[:], in_=winb[:, :])
            woutsb = constp.tile([128, 128], dt.bfloat16)
            nc.sync.dma_start(out=woutsb[:], in_=woutb[:, :])
            wdsb = constp.tile([128, 128], dt.bfloat16)
            nc.sync.dma_start(out=wdsb[:], in_=wdb[:, :])
            xT_sb = constp.tile([128, p.NA_TAIL], dt.float32)
            nc.sync.dma_start(out=xT_sb[:], in_=xT[:, :])
            convT = constp.tile([128, p.NA_TAIL], dt.bfloat16)

            conv_tiles = {}

            # ---- edge pipeline: 3-deep software pipeline ----
            # Iteration `it` emits: loads+phase1(it), phase2(it-1),
            # phase3a(it-2), S-load(it-2), phase3b(it-3).  Every cross-
            # engine dependency gets >= 1 full group of slack, so neither
            # the PE nor the ACT queue head ever waits on fresh results.
            # dijk on the SP HWDGE ring; xgT/S via SWDGE on idle GpSimd.
            t1s_q, wt_q, xg_q, wfT_q, sg_q = {}, {}, {}, {}, {}

            def emit_loads_p1(g):
                e0 = g * GRP
                dg = []
                for kc in range(p.NKC):
                    kn = p.KC[kc]
                    tdg = dload.tile([kn, GRP], dt.bfloat16,
                                     tag=f"dg{kc}", name=f"dg{kc}")
                    nc.sync.dma_start(out=tdg[:], in_=dsrc[kc][:, e0:e0 + GRP])
                    dg.append(tdg)
                xgg = dload.tile([128, GRP], dt.bfloat16, tag="xgg", bufs=3)
                nc.gpsimd.dma_start(out=xgg[:], in_=xgT[:, e0:e0 + GRP])
                xg_q[g] = xgg
                e1g = stage.tile([128, 4, BLK], dt.float32, tag="e1g")
                for b in range(4):
                    t1 = ps_t1.tile([128, BLK], dt.float32, tag="t1")
                    for kc in range(p.NKC):
                        kn = p.KC[kc]
                        nc.tensor.matmul(
                            t1[:], w1sb[kc][:kn, :],
                            dg[kc][:, b * BLK:(b + 1) * BLK],
                            start=(kc == 0), stop=(kc == p.NKC - 1),
                        )
                    nc.scalar.activation(e1g[:, b, :], t1[:], AF.Exp)
                t1sg = stage.tile([128, 4, BLK], dt.bfloat16, tag="t1sg",
                                  bufs=3)
                nc.scalar.activation(
                    t1sg[:], e1g[:], AF.Ln, bias=half_c[:], scale=half_c[:])
                t1s_q[g] = t1sg

            def emit_phase2(g):
                t1sg = t1s_q.pop(g)
                ewg = stage.tile([128, 4, BLK], dt.float32, tag="ewg")
                for b in range(4):
                    wps = ps_w.tile([128, BLK], dt.float32, tag="wps")
                    nc.tensor.matmul(
                        wps[:], w2sb[:], t1sg[:, b, :], start=True, stop=True)
                    nc.scalar.activation(ewg[:, b, :], wps[:], AF.Exp)
                wtg = stage.tile([128, 4, BLK], dt.float32, tag="wtg",
                                 bufs=3)
                nc.scalar.activation(
                    wtg[:], ewg[:], AF.Ln, bias=half_c[:], scale=half_c[:])
                wt_q[g] = wtg

            def emit_phase3a(g):
                wtg = wt_q.pop(g)
                xgg = xg_q.pop(g)
                wfT_g = []
                for b in range(4):
                    fps = ps_f.tile([128, BLK], dt.float32, tag="fps")
                    nc.tensor.matmul(
                        fps[:], winsb[:], xgg[:, b * BLK:(b + 1) * BLK],
                        start=True, stop=True)
                    wfT = work.tile([128, BLK], dt.bfloat16, tag="wfT",
                                    bufs=8)
                    nc.vector.tensor_tensor(
                        wfT[:], wtg[:, b, :], fps[:], mybir.AluOpType.mult)
                    wfT_g.append(wfT)
                wfT_q[g] = wfT_g

            def emit_sg_load(g):
                sg = dload.tile([128, 16, CHUNK_A], dt.bfloat16, tag="sg")
                nc.gpsimd.dma_start(
                    out=sg[:],
                    in_=s_pages[g * 16:(g + 1) * 16, :, :].rearrange(
                        "t pp c -> pp t c", pp=128),
                )
                sg_q[g] = sg

            def emit_phase3b(g):
                wfT_list = wfT_q.pop(g)
                sgq = sg_q.pop(g)
                for b in range(4):
                    wfT = wfT_list[b]
                    wfP = ps_tr.tile([128, BLK], dt.bfloat16, tag="wfP",
                                     name="wfP")
                    for i in range(4):
                        nc.tensor.transpose(
                            wfP[:, i * 128:(i + 1) * 128],
                            wfT[:, i * 128:(i + 1) * 128], idn[:])
                    wf = work.tile([128, BLK], dt.bfloat16, tag="wf",
                                   name="wf")
                    nc.vector.tensor_copy(wf[:], wfP[:])
                    for i in range(4):
                        t = g * 16 + b * 4 + i           # global tile id
                        k = p.tile_chunk[t]
                        if p.first_tile[k] == t:
                            cv = ps_cv.tile([128, CHUNK_A], dt.float32,
                                            tag="cv", name="cv")
                            nc.vector.memset(cv[:], 0.0)
                            conv_tiles[k] = cv
                        cv = conv_tiles[k]
                        nc.tensor.matmul(
                            cv[:], wf[:, i * 128:(i + 1) * 128],
                            sgq[:, b * 4 + i, :],
                            start=False, stop=(p.last_tile[k] == t),
                            skip_group_check=True,
                        )
                        if p.last_tile[k] == t:
                            nc.vector.tensor_copy(
                                convT[:, k * CHUNK_A:(k + 1) * CHUNK_A],
                                cv[:])
                            del conv_tiles[k]

            for it in range(p.NGRP + 3):
                if it < p.NGRP:
                    emit_loads_p1(it)
                if 1 <= it < p.NGRP + 1:
                    emit_phase2(it - 1)
                if 2 <= it < p.NGRP + 2:
                    emit_sg_load(it - 2)
                    emit_phase3a(it - 2)
                if 3 <= it < p.NGRP + 3:
                    emit_phase3b(it - 3)

            # ---- tail: z3^T = Wout.T @ conv^T, ssp, v^T, y^T ----
            for s in range(p.NSLAB):
                a0 = s * 512
                z3 = ps_t1.tile([128, 512], dt.float32, tag="t1", name="z3")
                nc.tensor.matmul(
                    z3[:], woutsb[:], convT[:, a0:a0 + 512], start=True, stop=True)
                e3 = tailp.tile([128, 512], dt.float32, tag="e3")
                nc.scalar.activation(e3[:], z3[:], AF.Exp)
                hT = tailp.tile([128, 512], dt.bfloat16, tag="hT")
                nc.scalar.activation(
                    hT[:], e3[:], AF.Ln, bias=half_c[:], scale=half_c[:])
                vps = ps_w.tile([128, 512], dt.float32, tag="wps", name="vps")
                nc.tensor.matmul(vps[:], wdsb[:], hT[:], start=True, stop=True)
                v_sb = tailp.tile([128, 512], dt.float32, tag="v_sb")
                nc.vector.tensor_copy(v_sb[:], vps[:])
                nc.sync.dma_start(out=v_out[:, a0:a0 + 512], in_=v_sb[:])
                y_sb = tailp.tile([128, 512], dt.float32, tag="y_sb")
                nc.vector.tensor_tensor(
                    y_sb[:], vps[:], xT_sb[:, a0:a0 + 512], mybir.AluOpType.add)
                nc.sync.dma_start(out=y_out[:, a0:a0 + 512], in_=y_sb[:])

    nc.finalize()
    return nc


_PROG_CACHE = {}


def kernel(x, dijk, W1, b1, W2, b2, Win, Wout, bout, Wd, bd, idx_j, seg_i, seg_j):
    x = np.ascontiguousarray(np.asarray(x, dtype=np.float32))
    dijk = np.ascontiguousarray(np.asarray(dijk, dtype=np.float32))
    for b in (b1, b2, bout, bd):
        assert np.abs(np.asarray(b)).max() == 0.0, "nonzero biases unsupported"

    n_atoms, n_basis = x.shape
    n_edges, n_in = dijk.shape
    assert n_basis == 128 and np.asarray(W2).shape == (128, 128)

    p = Plan(n_atoms, n_edges, n_in, seg_i)

    # global host-side layout transforms (shared across cores)
    dijk_bf_T = np.zeros((n_in, n_edges + 1), dtype=BF16)
    dijk_bf_T[:, :n_edges] = _to_bf16(dijk).T
    x_bf = _to_bf16(x)
    idx = np.asarray(idx_j).astype(np.int64)
    xgT_all = np.zeros((128, n_edges + 1), dtype=BF16)
    xgT_all[:, :n_edges] = x_bf[idx].T

    per_core = shard_inputs(p, x, dijk_bf_T, xgT_all, seg_i)
    del dijk_bf_T, xgT_all

    key = (n_atoms, n_edges, n_in, tuple(p.TPW))
    if key not in _PROG_CACHE:
        _PROG_CACHE[key] = build_program(p)
    nc = _PROG_CACHE[key]

    common = dict(
        w1b=_to_bf16(np.asarray(W1, dtype=np.float32)),
        w2b=_to_bf16(np.asarray(W2, dtype=np.float32)),
        winb=_to_bf16(np.asarray(Win, dtype=np.float32)),
        woutb=_to_bf16(np.asarray(Wout, dtype=np.float32)),
        wdb=_to_bf16(np.asarray(Wd, dtype=np.float32)),
        identb=_to_bf16(np.eye(128, dtype=np.float32)),
    )
    in_maps = [{**common, **pc} for pc in per_core]
    res = run_bass_kernel_spmd(nc, in_maps, list(range(N_CORES)))
    global LAST_RESULTS
    LAST_RESULTS = res

    y = np.empty((n_atoms, 128), dtype=np.float32)
    v = np.empty((n_atoms, 128), dtype=np.float32)
    for c in range(N_CORES):
        y[c * p.NA:(c + 1) * p.NA] = res.results[c]["y_out"][:, : p.NA].T
        v[c * p.NA:(c + 1) * p.NA] = res.results[c]["v_out"][:, : p.NA].T
    return (y, v)


# revision 22
# speedup vs baseline: 1.4844x; 1.0018x over previous
"""CFNet interaction block on 8 trn2 NeuronCores (SPMD bass/tile kernel), v2.

Per core c of 8 (SPMD, one program, per-core data): core c owns atoms
[c*NA, (c+1)*NA) and the edges whose sorted seg_i lands there.

Host prep = pure layout (no reference FLOPs):
  - dijk cast fp32->bf16 and pre-TRANSPOSED into k-chunks [128|128|44, E_PC]
    (kills the device-side PE transposes and halves dijk HBM traffic),
  - x rows pre-gathered by idx_j, transposed: xg^T [128x, E_PC] bf16
    (kills the v1 per-edge dma_gather: ~9 ns of Q7 per edge),
  - one-hot S pages [T, 128, 128] bf16: edge row -> column (atom mod 128).

Static SPMD schedule: local atoms are split into 128-atom chunks; chunk k
gets a FIXED tile allotment TPW[k] (mean + 6 sigma), so every core's tile t
maps to the same chunk and the same psum window -- per-core variation is
absorbed by padding (~7% edge inflation).  Pad edges carry all-zero S rows.

Device pipeline per 512-edge block, [feature, edge] layout (weights are the
matmul stationaries):
  mm1  t1 = W1c.T @ dT (3 k-chunks)          psum [f1, e]
  ssp1 Exp (psum->sbuf, per block), Ln(0.5x+0.5) batched [128, 2048] -> bf16
  mm2  w^T = W2.T @ t1s                      psum [f2, e]
  sspw Exp per block, Ln batched             -> wt^T fp32
  mm_f f^T = Win.T @ xg^T                    psum [f, e]
  wf^T = wt^T * f^T (DVE)                    -> bf16
  PE-transpose wf^T -> wf [e, f] (psum bf16 -> sbuf)
  mm3 per 128-edge tile: conv^T[chunk] += wf_tile.T @ S_tile
       into a [128, 128] psum window per chunk; ~49 flushes to sbuf.
A single manual InstLoadActFuncSet(natural_log_exp_and_others) keeps Exp+Ln
resident: no ACT_TABLE_LOAD thrash (was 2.6 ms in v1).

Tail from sbuf-resident conv^T: z3^T = Wout.T @ conv^T, ssp, v^T = Wd.T @
h^T, y^T = v^T + x^T.  Outputs leave TRANSPOSED [128, NA_PAD]; the host
transposes back.  ssp(x) = Ln(0.5 + 0.5*Exp(x)) exactly.
"""

import math
import sys

import numpy as np
import ml_dtypes

sys.path.insert(0, "/opt/trn_rl_repo")

import concourse.bacc as bacc
import concourse.mybir as mybir
from concourse import tile
from concourse.bass_utils import run_bass_kernel_spmd

dt = mybir.dt
AF = mybir.ActivationFunctionType
BF16 = ml_dtypes.bfloat16

N_CORES = 8
TILE_E = 128            # edges per S tile / conv matmul
BLK = 512               # edges per pipeline block
GRP = 2048              # edges per DMA group (4 blocks, 16 tiles)
CHUNK_A = 128           # atoms per conv psum window
ACT_SET_LN_EXP = 6      # natural_log_exp_and_others in act_info.json


def _ceil(a, b):
    return -(-a // b)


def _to_bf16(a):
    """fp32 -> bf16 with round-to-nearest-even, fast numpy path."""
    a = np.ascontiguousarray(a, dtype=np.float32)
    v = a.view(np.uint32)
    r = ((v + np.uint32(0x7FFF) + ((v >> np.uint32(16)) & np.uint32(1)))
         >> np.uint32(16)).astype(np.uint16)
    return r.view(BF16).reshape(a.shape)


class Plan:
    """Structure constants; identical for every core.  The per-chunk tile
    allotment TPW is fitted to the ACTUAL seg_i data (max over cores), so
    padding is just tile rounding (~3%) and fits by construction."""

    def __init__(self, n_atoms, n_edges, n_in, seg_i):
        assert n_atoms % N_CORES == 0
        self.n_atoms, self.n_edges, self.n_in = n_atoms, n_edges, n_in
        self.NA = n_atoms // N_CORES
        self.NCHUNK_REAL = _ceil(self.NA, CHUNK_A)
        seg_i = np.asarray(seg_i).astype(np.int64)
        bounds = np.searchsorted(seg_i, np.arange(N_CORES + 1) * self.NA)
        mx = np.zeros(self.NCHUNK_REAL, dtype=np.int64)
        for c in range(N_CORES):
            es = seg_i[bounds[c]:bounds[c + 1]] - c * self.NA
            cnt = np.bincount(es // CHUNK_A, minlength=self.NCHUNK_REAL)
            mx = np.maximum(mx, cnt)
        tpw = [int(_ceil(int(m), TILE_E)) for m in mx]
        t_raw = sum(tpw)
        self.T = int(_ceil(t_raw, GRP // TILE_E) * (GRP // TILE_E))
        self.TPW = tpw
        self.E_PC = self.T * TILE_E
        self.NBLK = self.E_PC // BLK
        self.NGRP = self.E_PC // GRP
        self.KC = [min(128, n_in - 128 * i) for i in range(_ceil(n_in, 128))]
        self.NKC = len(self.KC)
        # tile -> chunk map; trailing pad tiles attach to the last chunk
        self.first_tile = []
        self.tile_chunk = []
        for k, n in enumerate(tpw):
            self.first_tile.append(len(self.tile_chunk))
            self.tile_chunk += [k] * n
        self.tile_chunk += [self.NCHUNK_REAL - 1] * (self.T - t_raw)
        self.last_tile = [0] * self.NCHUNK_REAL
        for t, k in enumerate(self.tile_chunk):
            self.last_tile[k] = t
        self.NA_PAD = self.NCHUNK_REAL * CHUNK_A
        self.NSLAB = _ceil(self.NA_PAD, 512)         # tail slabs of 512 atoms
        self.NA_TAIL = self.NSLAB * 512


def shard_inputs(p, x, dijk_bf_T, xgT_all, seg_i):
    """Per-core layout prep. dijk_bf_T/xgT_all carry a zero pad column at
    index n_edges."""
    seg_i = np.asarray(seg_i).astype(np.int64)
    bounds = np.searchsorted(seg_i, np.arange(N_CORES + 1) * p.NA)
    ZCOL = p.n_edges                                  # the zero column

    per_core = []
    for c in range(N_CORES):
        lo, hi = int(bounds[c]), int(bounds[c + 1])
        es = seg_i[lo:hi] - c * p.NA                  # local atoms, sorted
        chunk = es // CHUNK_A

        # per-chunk edge placement into the static tile schedule
        cols = np.full(p.E_PC, ZCOL, dtype=np.int64)  # global edge id or pad
        s_t = np.empty(hi - lo, dtype=np.int64)       # tile of each edge
        s_r = np.empty(hi - lo, dtype=np.int64)       # row within tile
        cnt = np.bincount(chunk, minlength=p.NCHUNK_REAL)
        for k in range(p.NCHUNK_REAL):
            n_k = int(cnt[k])
            if n_k == 0:
                continue
            assert n_k <= p.TPW[k] * TILE_E, (c, k, n_k, p.TPW[k] * TILE_E)
            e0 = int(np.searchsorted(chunk, k))
            base = p.first_tile[k] * TILE_E
            pos = base + np.arange(n_k)
            cols[pos] = lo + e0 + np.arange(n_k)
            s_t[e0:e0 + n_k] = pos // TILE_E
            s_r[e0:e0 + n_k] = pos % TILE_E

        d = dijk_bf_T[:, cols]                        # [n_in, E_PC]
        d0 = np.ascontiguousarray(d[0:128])
        d1 = np.ascontiguousarray(d[128:256])
        d2 = np.ascontiguousarray(d[256:])
        xgT = np.ascontiguousarray(xgT_all[:, cols])  # [128, E_PC]

        S = np.zeros((p.T, TILE_E, CHUNK_A), dtype=BF16)
        s_c = es - chunk * CHUNK_A
        S[s_t, s_r, s_c] = 1.0

        xT = np.zeros((128, p.NA_TAIL), dtype=np.float32)
        xT[:, : p.NA] = x[c * p.NA : (c + 1) * p.NA].T

        per_core.append(dict(d0=d0, d1=d1, d2=d2, xgT=xgT, s_pages=S, xT=xT))
    return per_core


def build_program(p):
    nc = bacc.Bacc(None, target_bir_lowering=False)

    d0 = nc.declare_dram_parameter("d0", [128, p.E_PC], dt.bfloat16, isOutput=False)
    d1 = nc.declare_dram_parameter("d1", [128, p.E_PC], dt.bfloat16, isOutput=False)
    d2 = nc.declare_dram_parameter("d2", [p.KC[2], p.E_PC], dt.bfloat16, isOutput=False)
    xgT = nc.declare_dram_parameter("xgT", [128, p.E_PC], dt.bfloat16, isOutput=False)
    s_pages = nc.declare_dram_parameter(
        "s_pages", [p.T, TILE_E, CHUNK_A], dt.bfloat16, isOutput=False)
    xT = nc.declare_dram_parameter("xT", [128, p.NA_TAIL], dt.float32, isOutput=False)
    w1b = nc.declare_dram_parameter("w1b", [p.n_in, 128], dt.bfloat16, isOutput=False)
    w2b = nc.declare_dram_parameter("w2b", [128, 128], dt.bfloat16, isOutput=False)
    winb = nc.declare_dram_parameter("winb", [128, 128], dt.bfloat16, isOutput=False)
    woutb = nc.declare_dram_parameter("woutb", [128, 128], dt.bfloat16, isOutput=False)
    wdb = nc.declare_dram_parameter("wdb", [128, 128], dt.bfloat16, isOutput=False)
    identb = nc.declare_dram_parameter("identb", [128, 128], dt.bfloat16, isOutput=False)

    y_out = nc.declare_dram_parameter("y_out", [128, p.NA_TAIL], dt.float32, isOutput=True)
    v_out = nc.declare_dram_parameter("v_out", [128, p.NA_TAIL], dt.float32, isOutput=True)

    dsrc = [d0, d1, d2]

    with tile.TileContext(nc) as tc:
        # keep both Exp and Ln tables resident for the whole program
        nc.scalar.add_instruction(
            mybir.InstLoadActFuncSet(
                name=nc.get_next_instruction_name(), ins=[], outs=[],
                act_func_set_id=ACT_SET_LN_EXP,
            )
        )
        with (
            tc.tile_pool(name="const", bufs=1) as constp,
            tc.tile_pool(name="dload", bufs=2) as dload,
            tc.tile_pool(name="stage", bufs=2) as stage,
            tc.tile_pool(name="work", bufs=2) as work,
            tc.tile_pool(name="tail", bufs=2) as tailp,
            tc.tile_pool(name="ps_t1", bufs=2, space="PSUM") as ps_t1,
            tc.tile_pool(name="ps_w", bufs=2, space="PSUM") as ps_w,
            tc.tile_pool(name="ps_f", bufs=1, space="PSUM") as ps_f,
            tc.tile_pool(name="ps_tr", bufs=2, space="PSUM") as ps_tr,
            tc.tile_pool(name="ps_cv", bufs=1, space="PSUM") as ps_cv,
        ):
            # ---- constants ----
            idn = constp.tile([128, 128], dt.bfloat16)
            nc.sync.dma_start(out=idn[:], in_=identb[:, :])
            half_c = constp.tile([128, 1], dt.float32)
            nc.gpsimd.memset(half_c[:], 0.5)
            w1sb = []
            for kc in range(p.NKC):
                kn = p.KC[kc]
                t = constp.tile([128, 128], dt.bfloat16, name=f"w1sb{kc}")
                nc.sync.dma_start(out=t[:kn, :], in_=w1b[kc * 128: kc * 128 + kn, :])
                w1sb.append(t)
            w2sb = constp.tile([128, 128], dt.bfloat16)
            nc.sync.dma_start(out=w2sb[:], in_=w2b[:, :])
            winsb = constp.tile([128, 128], dt.bfloat16)
            nc.sync.dma_start(out=winsb[:], in_=winb[:, :])
            woutsb = constp.tile([128, 128], dt.bfloat16)
            nc.sync.dma_start(out=woutsb[:], in_=woutb[:, :])
            wdsb = constp.tile([128, 128], dt.bfloat16)
            nc.sync.dma_start(out=wdsb[:], in_=wdb[:, :])
            xT_sb = constp.tile([128, p.NA_TAIL], dt.float32)
            nc.sync.dma_start(out=xT_sb[:], in_=xT[:, :])
            convT = constp.tile([128, p.NA_TAIL], dt.bfloat16)

            conv_tiles = {}

            # ---- edge pipeline: 3-deep software pipeline ----
            # Iteration `it` emits: loads+phase1(it), phase2(it-1),
            # phase3a(it-2), S-load(it-2), phase3b(it-3).  Every cross-
            # engine dependency gets >= 1 full group of slack, so neither
            # the PE nor the ACT queue head ever waits on fresh results.
            # dijk on the SP HWDGE ring; xgT/S via SWDGE on idle GpSimd.
            t1s_q, wt_q, xg_q, wfT_q, sg_q = {}, {}, {}, {}, {}

            def emit_loads_p1(g):
                e0 = g * GRP
                dg = []
                for kc in range(p.NKC):
                    kn = p.KC[kc]
                    tdg = dload.tile([kn, GRP], dt.bfloat16,
                                     tag=f"dg{kc}", name=f"dg{kc}")
                    nc.sync.dma_start(out=tdg[:], in_=dsrc[kc][:, e0:e0 + GRP])
                    dg.append(tdg)
                xgg = dload.tile([128, GRP], dt.bfloat16, tag="xgg", bufs=3)
                nc.gpsimd.dma_start(out=xgg[:], in_=xgT[:, e0:e0 + GRP])
                xg_q[g] = xgg
                e1g = stage.tile([128, 4, BLK], dt.float32, tag="e1g")
                for b in range(4):
                    t1 = ps_t1.tile([128, BLK], dt.float32, tag="t1")
                    for kc in range(p.NKC):
                        kn = p.KC[kc]
                        nc.tensor.matmul(
                            t1[:], w1sb[kc][:kn, :],
                            dg[kc][:, b * BLK:(b + 1) * BLK],
                            start=(kc == 0), stop=(kc == p.NKC - 1),
                        )
                    nc.scalar.activation(e1g[:, b, :], t1[:], AF.Exp)
                t1sg = stage.tile([128, 4, BLK], dt.bfloat16, tag="t1sg",
                                  bufs=3)
                nc.scalar.activation(
                    t1sg[:], e1g[:], AF.Ln, bias=half_c[:], scale=half_c[:])
                t1s_q[g] = t1sg

            def emit_phase2(g):
                t1sg = t1s_q.pop(g)
                ewg = stage.tile([128, 4, BLK], dt.float32, tag="ewg")
                for b in range(4):
                    wps = ps_w.tile([128, BLK], dt.float32, tag="wps")
                    nc.tensor.matmul(
                        wps[:], w2sb[:], t1sg[:, b, :], start=True, stop=True)
                    nc.scalar.activation(ewg[:, b, :], wps[:], AF.Exp)
                wtg = stage.tile([128, 4, BLK], dt.float32, tag="wtg",
                                 bufs=3)
                nc.scalar.activation(
                    wtg[:], ewg[:], AF.Ln, bias=half_c[:], scale=half_c[:])
                wt_q[g] = wtg

            def emit_phase3a(g):
                wtg = wt_q.pop(g)
                xgg = xg_q.pop(g)
                wfT_g = []
                for b in range(4):
                    fps = ps_f.tile([128, BLK], dt.float32, tag="fps")
                    nc.tensor.matmul(
                        fps[:], winsb[:], xgg[:, b * BLK:(b + 1) * BLK],
                        start=True, stop=True)
                    wfT = work.tile([128, BLK], dt.bfloat16, tag="wfT",
                                    bufs=8)
                    nc.vector.tensor_tensor(
                        wfT[:], wtg[:, b, :], fps[:], mybir.AluOpType.mult)
                    wfT_g.append(wfT)
                wfT_q[g] = wfT_g

            def emit_sg_load(g):
                sg = dload.tile([128, 16, CHUNK_A], dt.bfloat16, tag="sg")
                nc.gpsimd.dma_start(
                    out=sg[:],
                    in_=s_pages[g * 16:(g + 1) * 16, :, :].rearrange(
                        "t pp c -> pp t c", pp=128),
                )
                sg_q[g] = sg

            def emit_phase3b(g):
                wfT_list = wfT_q.pop(g)
                sgq = sg_q.pop(g)
                for b in range(4):
                    wfT = wfT_list[b]
                    wfP = ps_tr.tile([128, BLK], dt.bfloat16, tag="wfP",
                                     name="wfP")
                    for i in range(4):
                        nc.tensor.transpose(
                            wfP[:, i * 128:(i + 1) * 128],
                            wfT[:, i * 128:(i + 1) * 128], idn[:])
                    wf = work.tile([128, BLK], dt.bfloat16, tag="wf",
                                   name="wf")
                    nc.vector.tensor_copy(wf[:], wfP[:])
                    for i in range(4):
                        t = g * 16 + b * 4 + i           # global tile id
                        k = p.tile_chunk[t]
                        if p.first_tile[k] == t:
                            cv = ps_cv.tile([128, CHUNK_A], dt.float32,
                                            tag="cv", name="cv")
                            nc.vector.memset(cv[:], 0.0)
                            conv_tiles[k] = cv
                        cv = conv_tiles[k]
                        nc.tensor.matmul(
                            cv[:], wf[:, i * 128:(i + 1) * 128],
                            sgq[:, b * 4 + i, :],
                            start=False, stop=(p.last_tile[k] == t),
                            skip_group_check=True,
                        )
                        if p.last_tile[k] == t:
                            nc.vector.tensor_copy(
                                convT[:, k * CHUNK_A:(k + 1) * CHUNK_A],
                                cv[:])
                            del conv_tiles[k]

            for it in range(p.NGRP + 3):
                if it < p.NGRP:
                    emit_loads_p1(it)
                if 1 <= it < p.NGRP + 1:
                    emit_phase2(it - 1)
                if 2 <= it < p.NGRP + 2:
                    emit_sg_load(it - 2)
                    emit_phase3a(it - 2)
                if 3 <= it < p.NGRP + 3:
                    emit_phase3b(it - 3)

            # ---- tail: z3^T = Wout.T @ conv^T, ssp, v^T, y^T ----
            for s in range(p.NSLAB):
                a0 = s * 512
                z3 = ps_t1.tile([128, 512], dt.float32, tag="t1", name="z3")
                nc.tensor.matmul(
                    z3[:], woutsb[:], convT[:, a0:a0 + 512], start=True, stop=True)
                e3 = tailp.tile([128, 512], dt.float32, tag="e3")
                nc.scalar.activation(e3[:], z3[:], AF.Exp)
                hT = tailp.tile([128, 512], dt.bfloat16, tag="hT")
                nc.scalar.activation(
                    hT[:], e3[:], AF.Ln, bias=half_c[:], scale=half_c[:])
                vps = ps_w.tile([128, 512], dt.float32, tag="wps", name="vps")
                nc.tensor.matmul(vps[:], wdsb[:], hT[:], start=True, stop=True)
                v_sb = tailp.tile([128, 512], dt.float32, tag="v_sb")
                nc.vector.tensor_copy(v_sb[:], vps[:])
                nc.sync.dma_start(out=v_out[:, a0:a0 + 512], in_=v_sb[:])
                y_sb = tailp.tile([128, 512], dt.float32, tag="y_sb")
                nc.vector.tensor_tensor(
                    y_sb[:], vps[:], xT_sb[:, a0:a0 + 512], mybir.AluOpType.add)
                nc.sync.dma_start(out=y_out[:, a0:a0 + 512], in_=y_sb[:])

    nc.finalize()
    return nc


_PROG_CACHE = {}


def kernel(x, dijk, W1, b1, W2, b2, Win, Wout, bout, Wd, bd, idx_j, seg_i, seg_j):
    x = np.ascontiguousarray(np.asarray(x, dtype=np.float32))
    dijk = np.ascontiguousarray(np.asarray(dijk, dtype=np.float32))
    for b in (b1, b2, bout, bd):
        assert np.abs(np.asarray(b)).max() == 0.0, "nonzero biases unsupported"

    n_atoms, n_basis = x.shape
    n_edges, n_in = dijk.shape
    assert n_basis == 128 and np.asarray(W2).shape == (128, 128)

    p = Plan(n_atoms, n_edges, n_in, seg_i)

    # global host-side layout transforms (shared across cores)
    dijk_bf_T = np.zeros((n_in, n_edges + 1), dtype=BF16)
    dijk_bf_T[:, :n_edges] = _to_bf16(dijk).T
    x_bf = _to_bf16(x)
    idx = np.asarray(idx_j).astype(np.int64)
    xgT_all = np.zeros((128, n_edges + 1), dtype=BF16)
    xgT_all[:, :n_edges] = x_bf[idx].T

    per_core = shard_inputs(p, x, dijk_bf_T, xgT_all, seg_i)
    del dijk_bf_T, xgT_all

    key = (n_atoms, n_edges, n_in, tuple(p.TPW))
    if key not in _PROG_CACHE:
        _PROG_CACHE[key] = build_program(p)
    nc = _PROG_CACHE[key]

    common = dict(
        w1b=_to_bf16(np.asarray(W1, dtype=np.float32)),
        w2b=_to_bf16(np.asarray(W2, dtype=np.float32)),
        winb=_to_bf16(np.asarray(Win, dtype=np.float32)),
        woutb=_to_bf16(np.asarray(Wout, dtype=np.float32)),
        wdb=_to_bf16(np.asarray(Wd, dtype=np.float32)),
        identb=_to_bf16(np.eye(128, dtype=np.float32)),
    )
    in_maps = [{**common, **pc} for pc in per_core]
    res = run_bass_kernel_spmd(nc, in_maps, list(range(N_CORES)))
    global LAST_RESULTS
    LAST_RESULTS = res

    y = np.empty((n_atoms, 128), dtype=np.float32)
    v = np.empty((n_atoms, 128), dtype=np.float32)
    for c in range(N_CORES):
        y[c * p.NA:(c + 1) * p.NA] = res.results[c]["y_out"][:, : p.NA].T
        v[c * p.NA:(c + 1) * p.NA] = res.results[c]["v_out"][:, : p.NA].T
    return (y, v)


# revision 27
# speedup vs baseline: 1.5092x; 1.0167x over previous
"""CFNet interaction block on 8 trn2 NeuronCores (SPMD bass/tile kernel), v2.

Per core c of 8 (SPMD, one program, per-core data): core c owns atoms
[c*NA, (c+1)*NA) and the edges whose sorted seg_i lands there.

Host prep = pure layout (no reference FLOPs):
  - dijk cast fp32->bf16 and pre-TRANSPOSED into k-chunks [128|128|44, E_PC]
    (kills the device-side PE transposes and halves dijk HBM traffic),
  - x rows pre-gathered by idx_j, transposed: xg^T [128x, E_PC] bf16
    (kills the v1 per-edge dma_gather: ~9 ns of Q7 per edge),
  - one-hot S pages [T, 128, 128] bf16: edge row -> column (atom mod 128).

Static SPMD schedule: local atoms are split into 128-atom chunks; chunk k
gets a FIXED tile allotment TPW[k] (mean + 6 sigma), so every core's tile t
maps to the same chunk and the same psum window -- per-core variation is
absorbed by padding (~7% edge inflation).  Pad edges carry all-zero S rows.

Device pipeline per 512-edge block, [feature, edge] layout (weights are the
matmul stationaries):
  mm1  t1 = W1c.T @ dT (3 k-chunks)          psum [f1, e]
  ssp1 Exp (psum->sbuf, per block), Ln(0.5x+0.5) batched [128, 2048] -> bf16
  mm2  w^T = W2.T @ t1s                      psum [f2, e]
  sspw Exp per block, Ln batched             -> wt^T fp32
  mm_f f^T = Win.T @ xg^T                    psum [f, e]
  wf^T = wt^T * f^T (DVE)                    -> bf16
  PE-transpose wf^T -> wf [e, f] (psum bf16 -> sbuf)
  mm3 per 128-edge tile: conv^T[chunk] += wf_tile.T @ S_tile
       into a [128, 128] psum window per chunk; ~49 flushes to sbuf.
A single manual InstLoadActFuncSet(natural_log_exp_and_others) keeps Exp+Ln
resident: no ACT_TABLE_LOAD thrash (was 2.6 ms in v1).

Tail from sbuf-resident conv^T: z3^T = Wout.T @ conv^T, ssp, v^T = Wd.T @
h^T, y^T = v^T + x^T.  Outputs leave TRANSPOSED [128, NA_PAD]; the host
transposes back.  ssp(x) = Ln(0.5 + 0.5*Exp(x)) exactly.
"""

import math
import sys

import numpy as np
import ml_dtypes

sys.path.insert(0, "/opt/trn_rl_repo")

import concourse.bacc as bacc
import concourse.mybir as mybir
from concourse import tile
from concourse.bass_utils import run_bass_kernel_spmd

dt = mybir.dt
AF = mybir.ActivationFunctionType
BF16 = ml_dtypes.bfloat16

N_CORES = 8
TILE_E = 128            # edges per S tile / conv matmul
BLK = 512               # edges per pipeline block
GRP = 2048              # edges per DMA group (4 blocks, 16 tiles)
CHUNK_A = 128           # atoms per conv psum window
ACT_SET_LN_EXP = 6      # natural_log_exp_and_others in act_info.json


def _ceil(a, b):
    return -(-a // b)


def _to_bf16(a):
    """fp32 -> bf16 with round-to-nearest-even, fast numpy path."""
    a = np.ascontiguousarray(a, dtype=np.float32)
    v = a.view(np.uint32)
    r = ((v + np.uint32(0x7FFF) + ((v >> np.uint32(16)) & np.uint32(1)))
         >> np.uint32(16)).astype(np.uint16)
    return r.view(BF16).reshape(a.shape)


class Plan:
    """Structure constants; identical for every core.  The per-chunk tile
    allotment TPW is fitted to the ACTUAL seg_i data (max over cores), so
    padding is just tile rounding (~3%) and fits by construction."""

    def __init__(self, n_atoms, n_edges, n_in, seg_i):
        assert n_atoms % N_CORES == 0
        self.n_atoms, self.n_edges, self.n_in = n_atoms, n_edges, n_in
        self.NA = n_atoms // N_CORES
        self.NCHUNK_REAL = _ceil(self.NA, CHUNK_A)
        seg_i = np.asarray(seg_i).astype(np.int64)
        bounds = np.searchsorted(seg_i, np.arange(N_CORES + 1) * self.NA)
        mx = np.zeros(self.NCHUNK_REAL, dtype=np.int64)
        for c in range(N_CORES):
            es = seg_i[bounds[c]:bounds[c + 1]] - c * self.NA
            cnt = np.bincount(es // CHUNK_A, minlength=self.NCHUNK_REAL)
            mx = np.maximum(mx, cnt)
        tpw = [int(_ceil(int(m), TILE_E)) for m in mx]
        t_raw = sum(tpw)
        self.T = int(_ceil(t_raw, GRP // TILE_E) * (GRP // TILE_E))
        self.TPW = tpw
        self.E_PC = self.T * TILE_E
        self.NBLK = self.E_PC // BLK
        self.NGRP = self.E_PC // GRP
        self.KC = [min(128, n_in - 128 * i) for i in range(_ceil(n_in, 128))]
        self.NKC = len(self.KC)
        # tile -> chunk map; trailing pad tiles attach to the last chunk
        self.first_tile = []
        self.tile_chunk = []
        for k, n in enumerate(tpw):
            self.first_tile.append(len(self.tile_chunk))
            self.tile_chunk += [k] * n
        self.tile_chunk += [self.NCHUNK_REAL - 1] * (self.T - t_raw)
        self.last_tile = [0] * self.NCHUNK_REAL
        for t, k in enumerate(self.tile_chunk):
            self.last_tile[k] = t
        self.NA_PAD = self.NCHUNK_REAL * CHUNK_A
        self.NSLAB = _ceil(self.NA_PAD, 512)         # tail slabs of 512 atoms
        self.NA_TAIL = self.NSLAB * 512


def shard_inputs(p, x, dijk_bf_T, xgT_all, seg_i):
    """Per-core layout prep. dijk_bf_T/xgT_all carry a zero pad column at
    index n_edges."""
    seg_i = np.asarray(seg_i).astype(np.int64)
    bounds = np.searchsorted(seg_i, np.arange(N_CORES + 1) * p.NA)
    ZCOL = p.n_edges                                  # the zero column

    per_core = []
    for c in range(N_CORES):
        lo, hi = int(bounds[c]), int(bounds[c + 1])
        es = seg_i[lo:hi] - c * p.NA                  # local atoms, sorted
        chunk = es // CHUNK_A

        # per-chunk edge placement into the static tile schedule
        cols = np.full(p.E_PC, ZCOL, dtype=np.int64)  # global edge id or pad
        s_t = np.empty(hi - lo, dtype=np.int64)       # tile of each edge
        s_r = np.empty(hi - lo, dtype=np.int64)       # row within tile
        cnt = np.bincount(chunk, minlength=p.NCHUNK_REAL)
        for k in range(p.NCHUNK_REAL):
            n_k = int(cnt[k])
            if n_k == 0:
                continue
            assert n_k <= p.TPW[k] * TILE_E, (c, k, n_k, p.TPW[k] * TILE_E)
            e0 = int(np.searchsorted(chunk, k))
            base = p.first_tile[k] * TILE_E
            pos = base + np.arange(n_k)
            cols[pos] = lo + e0 + np.arange(n_k)
            s_t[e0:e0 + n_k] = pos // TILE_E
            s_r[e0:e0 + n_k] = pos % TILE_E

        d = dijk_bf_T[:, cols]                        # [n_in, E_PC]
        d0 = np.ascontiguousarray(d[0:128])
        d1 = np.ascontiguousarray(d[128:256])
        d2 = np.ascontiguousarray(d[256:])
        xgT = np.ascontiguousarray(xgT_all[:, cols])  # [128, E_PC]

        S = np.zeros((p.T, TILE_E, CHUNK_A), dtype=BF16)
        s_c = es - chunk * CHUNK_A
        S[s_t, s_r, s_c] = 1.0

        xT = np.zeros((128, p.NA_TAIL), dtype=np.float32)
        xT[:, : p.NA] = x[c * p.NA : (c + 1) * p.NA].T

        per_core.append(dict(d0=d0, d1=d1, d2=d2, xgT=xgT, s_pages=S, xT=xT))
    return per_core


def build_program(p):
    nc = bacc.Bacc(None, target_bir_lowering=False)

    d0 = nc.declare_dram_parameter("d0", [128, p.E_PC], dt.bfloat16, isOutput=False)
    d1 = nc.declare_dram_parameter("d1", [128, p.E_PC], dt.bfloat16, isOutput=False)
    d2 = nc.declare_dram_parameter("d2", [p.KC[2], p.E_PC], dt.bfloat16, isOutput=False)
    xgT = nc.declare_dram_parameter("xgT", [128, p.E_PC], dt.bfloat16, isOutput=False)
    s_pages = nc.declare_dram_parameter(
        "s_pages", [p.T, TILE_E, CHUNK_A], dt.bfloat16, isOutput=False)
    xT = nc.declare_dram_parameter("xT", [128, p.NA_TAIL], dt.float32, isOutput=False)
    w1b = nc.declare_dram_parameter("w1b", [p.n_in, 128], dt.bfloat16, isOutput=False)
    w2b = nc.declare_dram_parameter("w2b", [128, 128], dt.bfloat16, isOutput=False)
    winb = nc.declare_dram_parameter("winb", [128, 128], dt.bfloat16, isOutput=False)
    woutb = nc.declare_dram_parameter("woutb", [128, 128], dt.bfloat16, isOutput=False)
    wdb = nc.declare_dram_parameter("wdb", [128, 128], dt.bfloat16, isOutput=False)
    identb = nc.declare_dram_parameter("identb", [128, 128], dt.bfloat16, isOutput=False)

    y_out = nc.declare_dram_parameter("y_out", [128, p.NA_TAIL], dt.float32, isOutput=True)
    v_out = nc.declare_dram_parameter("v_out", [128, p.NA_TAIL], dt.float32, isOutput=True)

    dsrc = [d0, d1, d2]

    with tile.TileContext(nc) as tc:
        # keep both Exp and Ln tables resident for the whole program
        nc.scalar.add_instruction(
            mybir.InstLoadActFuncSet(
                name=nc.get_next_instruction_name(), ins=[], outs=[],
                act_func_set_id=ACT_SET_LN_EXP,
            )
        )
        with (
            tc.tile_pool(name="const", bufs=1) as constp,
            tc.tile_pool(name="dload", bufs=2) as dload,
            tc.tile_pool(name="stage", bufs=2) as stage,
            tc.tile_pool(name="work", bufs=2) as work,
            tc.tile_pool(name="tail", bufs=2) as tailp,
            tc.tile_pool(name="ps_t1", bufs=2, space="PSUM") as ps_t1,
            tc.tile_pool(name="ps_w", bufs=2, space="PSUM") as ps_w,
            tc.tile_pool(name="ps_f", bufs=1, space="PSUM") as ps_f,
            tc.tile_pool(name="ps_tr", bufs=2, space="PSUM") as ps_tr,
            tc.tile_pool(name="ps_cv", bufs=1, space="PSUM") as ps_cv,
        ):
            # ---- constants ----
            idn = constp.tile([128, 128], dt.bfloat16)
            nc.sync.dma_start(out=idn[:], in_=identb[:, :])
            half_c = constp.tile([128, 1], dt.float32)
            nc.gpsimd.memset(half_c[:], 0.5)
            w1sb = []
            for kc in range(p.NKC):
                kn = p.KC[kc]
                t = constp.tile([128, 128], dt.bfloat16, name=f"w1sb{kc}")
                nc.sync.dma_start(out=t[:kn, :], in_=w1b[kc * 128: kc * 128 + kn, :])
                w1sb.append(t)
            w2sb = constp.tile([128, 128], dt.bfloat16)
            nc.sync.dma_start(out=w2sb[:], in_=w2b[:, :])
            winsb = constp.tile([128, 128], dt.bfloat16)
            nc.sync.dma_start(out=winsb[:], in_=winb[:, :])
            woutsb = constp.tile([128, 128], dt.bfloat16)
            nc.sync.dma_start(out=woutsb[:], in_=woutb[:, :])
            wdsb = constp.tile([128, 128], dt.bfloat16)
            nc.sync.dma_start(out=wdsb[:], in_=wdb[:, :])
            xT_sb = constp.tile([128, p.NA_TAIL], dt.float32)
            nc.gpsimd.dma_start(out=xT_sb[:], in_=xT[:, :])
            convT = constp.tile([128, p.NA_TAIL], dt.bfloat16)

            conv_tiles = {}
            tail_done = set()

            def emit_tail_slab(s):
                # z3^T = Wout.T @ conv^T, ssp, v^T = Wd.T @ h^T, y^T = v + x
                tail_done.add(s)
                a0 = s * 512
                z3 = ps_t1.tile([128, 512], dt.float32, tag="t1", name="z3")
                nc.tensor.matmul(
                    z3[:], woutsb[:], convT[:, a0:a0 + 512],
                    start=True, stop=True)
                e3 = tailp.tile([128, 512], dt.float32, tag="e3")
                nc.scalar.activation(e3[:], z3[:], AF.Exp)
                hT = tailp.tile([128, 512], dt.bfloat16, tag="hT")
                nc.scalar.activation(
                    hT[:], e3[:], AF.Ln, bias=half_c[:], scale=half_c[:])
                vps = ps_w.tile([128, 512], dt.float32, tag="wps", name="vps")
                nc.tensor.matmul(vps[:], wdsb[:], hT[:], start=True, stop=True)
                v_sb = tailp.tile([128, 512], dt.float32, tag="v_sb")
                nc.vector.tensor_copy(v_sb[:], vps[:])
                nc.sync.dma_start(out=v_out[:, a0:a0 + 512], in_=v_sb[:])
                y_sb = tailp.tile([128, 512], dt.float32, tag="y_sb")
                nc.vector.tensor_tensor(
                    y_sb[:], vps[:], xT_sb[:, a0:a0 + 512],
                    mybir.AluOpType.add)
                nc.sync.dma_start(out=y_out[:, a0:a0 + 512], in_=y_sb[:])

            # ---- edge pipeline: 3-deep software pipeline ----
            # Iteration `it` emits: loads+phase1(it), phase2(it-1),
            # phase3a(it-2), S-load(it-2), phase3b(it-3).  Every cross-
            # engine dependency gets >= 1 full group of slack, so neither
            # the PE nor the ACT queue head ever waits on fresh results.
            # dijk on the SP HWDGE ring; xgT/S via SWDGE on idle GpSimd.
            t1s_q, wt_q, xg_q, wfT_q, sg_q = {}, {}, {}, {}, {}

            def emit_loads_p1(g):
                e0 = g * GRP
                dg = []
                for kc in range(p.NKC):
                    kn = p.KC[kc]
                    tdg = dload.tile([kn, GRP], dt.bfloat16,
                                     tag=f"dg{kc}", name=f"dg{kc}", bufs=3)
                    nc.sync.dma_start(out=tdg[:], in_=dsrc[kc][:, e0:e0 + GRP])
                    dg.append(tdg)
                xgg = dload.tile([128, GRP], dt.bfloat16, tag="xgg", bufs=4)
                nc.gpsimd.dma_start(out=xgg[:], in_=xgT[:, e0:e0 + GRP])
                xg_q[g] = xgg
                e1g = stage.tile([128, 4, BLK], dt.float32, tag="e1g")
                for b in range(4):
                    t1 = ps_t1.tile([128, BLK], dt.float32, tag="t1")
                    for kc in range(p.NKC):
                        kn = p.KC[kc]
                        nc.tensor.matmul(
                            t1[:], w1sb[kc][:kn, :],
                            dg[kc][:, b * BLK:(b + 1) * BLK],
                            start=(kc == 0), stop=(kc == p.NKC - 1),
                        )
                    nc.scalar.activation(e1g[:, b, :], t1[:], AF.Exp)
                t1sg = stage.tile([128, 4, BLK], dt.bfloat16, tag="t1sg",
                                  bufs=3)
                nc.scalar.activation(
                    t1sg[:], e1g[:], AF.Ln, bias=half_c[:], scale=half_c[:])
                t1s_q[g] = t1sg

            def emit_phase2(g):
                t1sg = t1s_q.pop(g)
                ewg = stage.tile([128, 4, BLK], dt.float32, tag="ewg")
                for b in range(4):
                    wps = ps_w.tile([128, BLK], dt.float32, tag="wps")
                    nc.tensor.matmul(
                        wps[:], w2sb[:], t1sg[:, b, :], start=True, stop=True)
                    nc.scalar.activation(ewg[:, b, :], wps[:], AF.Exp)
                wtg = stage.tile([128, 4, BLK], dt.float32, tag="wtg",
                                 bufs=3)
                nc.scalar.activation(
                    wtg[:], ewg[:], AF.Ln, bias=half_c[:], scale=half_c[:])
                wt_q[g] = wtg

            def emit_phase3a(g):
                wtg = wt_q.pop(g)
                xgg = xg_q.pop(g)
                wfT_g = []
                for b in range(4):
                    fps = ps_f.tile([128, BLK], dt.float32, tag="fps")
                    nc.tensor.matmul(
                        fps[:], winsb[:], xgg[:, b * BLK:(b + 1) * BLK],
                        start=True, stop=True)
                    wfT = work.tile([128, BLK], dt.bfloat16, tag="wfT",
                                    bufs=8)
                    nc.vector.tensor_tensor(
                        wfT[:], wtg[:, b, :], fps[:], mybir.AluOpType.mult)
                    wfT_g.append(wfT)
                wfT_q[g] = wfT_g

            def emit_sg_load(g):
                sg = dload.tile([128, 16, CHUNK_A], dt.bfloat16, tag="sg")
                nc.gpsimd.dma_start(
                    out=sg[:],
                    in_=s_pages[g * 16:(g + 1) * 16, :, :].rearrange(
                        "t pp c -> pp t c", pp=128),
                )
                sg_q[g] = sg

            def emit_phase3b(g):
                wfT_list = wfT_q.pop(g)
                sgq = sg_q.pop(g)
                for b in range(4):
                    wfT = wfT_list[b]
                    wfP = ps_tr.tile([128, BLK], dt.bfloat16, tag="wfP",
                                     name="wfP")
                    for i in range(4):
                        nc.tensor.transpose(
                            wfP[:, i * 128:(i + 1) * 128],
                            wfT[:, i * 128:(i + 1) * 128], idn[:])
                    wf = work.tile([128, BLK], dt.bfloat16, tag="wf",
                                   name="wf")
                    nc.vector.tensor_copy(wf[:], wfP[:])
                    for i in range(4):
                        t = g * 16 + b * 4 + i           # global tile id
                        k = p.tile_chunk[t]
                        if p.first_tile[k] == t:
                            cv = ps_cv.tile([128, CHUNK_A], dt.float32,
                                            tag="cv", name="cv")
                            nc.vector.memset(cv[:], 0.0)
                            conv_tiles[k] = cv
                        cv = conv_tiles[k]
                        nc.tensor.matmul(
                            cv[:], wf[:, i * 128:(i + 1) * 128],
                            sgq[:, b * 4 + i, :],
                            start=False, stop=(p.last_tile[k] == t),
                            skip_group_check=True,
                        )
                        if p.last_tile[k] == t:
                            nc.vector.tensor_copy(
                                convT[:, k * CHUNK_A:(k + 1) * CHUNK_A],
                                cv[:])
                            del conv_tiles[k]
                            # interleave tail slabs whose conv chunks are
                            # all flushed (flushes are in chunk order)
                            for s in range(p.NSLAB):
                                if (s not in tail_done
                                        and min(4 * s + 3,
                                                p.NCHUNK_REAL - 1) <= k):
                                    emit_tail_slab(s)

            for it in range(p.NGRP + 3):
                if it < p.NGRP:
                    emit_loads_p1(it)
                if 1 <= it < p.NGRP + 1:
                    emit_phase2(it - 1)
                if 2 <= it < p.NGRP + 2:
                    emit_sg_load(it - 2)
                    emit_phase3a(it - 2)
                if 3 <= it < p.NGRP + 3:
                    emit_phase3b(it - 3)

            # ---- tail safety sweep (slabs not emitted inline) ----
            for s in range(p.NSLAB):
                if s not in tail_done:
                    emit_tail_slab(s)

    nc.finalize()
    return nc


_PROG_CACHE = {}


def kernel(x, dijk, W1, b1, W2, b2, Win, Wout, bout, Wd, bd, idx_j, seg_i, seg_j):
    x = np.ascontiguousarray(np.asarray(x, dtype=np.float32))
    dijk = np.ascontiguousarray(np.asarray(dijk, dtype=np.float32))
    for b in (b1, b2, bout, bd):
        assert np.abs(np.asarray(b)).max() == 0.0, "nonzero biases unsupported"

    n_atoms, n_basis = x.shape
    n_edges, n_in = dijk.shape
    assert n_basis == 128 and np.asarray(W2).shape == (128, 128)

    p = Plan(n_atoms, n_edges, n_in, seg_i)

    # global host-side layout transforms (shared across cores)
    dijk_bf_T = np.zeros((n_in, n_edges + 1), dtype=BF16)
    dijk_bf_T[:, :n_edges] = _to_bf16(dijk).T
    x_bf = _to_bf16(x)
    idx = np.asarray(idx_j).astype(np.int64)
    xgT_all = np.zeros((128, n_edges + 1), dtype=BF16)
    xgT_all[:, :n_edges] = x_bf[idx].T

    per_core = shard_inputs(p, x, dijk_bf_T, xgT_all, seg_i)
    del dijk_bf_T, xgT_all

    key = (n_atoms, n_edges, n_in, tuple(p.TPW))
    if key not in _PROG_CACHE:
        _PROG_CACHE[key] = build_program(p)
    nc = _PROG_CACHE[key]

    common = dict(
        w1b=_to_bf16(np.asarray(W1, dtype=np.float32)),
        w2b=_to_bf16(np.asarray(W2, dtype=np.float32)),
        winb=_to_bf16(np.asarray(Win, dtype=np.float32)),
        woutb=_to_bf16(np.asarray(Wout, dtype=np.float32)),
        wdb=_to_bf16(np.asarray(Wd, dtype=np.float32)),
        identb=_to_bf16(np.eye(128, dtype=np.float32)),
    )
    in_maps = [{**common, **pc} for pc in per_core]
    res = run_bass_kernel_spmd(nc, in_maps, list(range(N_CORES)))
    global LAST_RESULTS
    LAST_RESULTS = res

    y = np.empty((n_atoms, 128), dtype=np.float32)
    v = np.empty((n_atoms, 128), dtype=np.float32)
    for c in range(N_CORES):
        y[c * p.NA:(c + 1) * p.NA] = res.results[c]["y_out"][:, : p.NA].T
        v[c * p.NA:(c + 1) * p.NA] = res.results[c]["v_out"][:, : p.NA].T
    return (y, v)


# revision 28
# speedup vs baseline: 1.5180x; 1.0058x over previous
"""CFNet interaction block on 8 trn2 NeuronCores (SPMD bass/tile kernel), v2.

Per core c of 8 (SPMD, one program, per-core data): core c owns atoms
[c*NA, (c+1)*NA) and the edges whose sorted seg_i lands there.

Host prep = pure layout (no reference FLOPs):
  - dijk cast fp32->bf16 and pre-TRANSPOSED into k-chunks [128|128|44, E_PC]
    (kills the device-side PE transposes and halves dijk HBM traffic),
  - x rows pre-gathered by idx_j, transposed: xg^T [128x, E_PC] bf16
    (kills the v1 per-edge dma_gather: ~9 ns of Q7 per edge),
  - one-hot S pages [T, 128, 128] bf16: edge row -> column (atom mod 128).

Static SPMD schedule: local atoms are split into 128-atom chunks; chunk k
gets a FIXED tile allotment TPW[k] (mean + 6 sigma), so every core's tile t
maps to the same chunk and the same psum window -- per-core variation is
absorbed by padding (~7% edge inflation).  Pad edges carry all-zero S rows.

Device pipeline per 512-edge block, [feature, edge] layout (weights are the
matmul stationaries):
  mm1  t1 = W1c.T @ dT (3 k-chunks)          psum [f1, e]
  ssp1 Exp (psum->sbuf, per block), Ln(0.5x+0.5) batched [128, 2048] -> bf16
  mm2  w^T = W2.T @ t1s                      psum [f2, e]
  sspw Exp per block, Ln batched             -> wt^T fp32
  mm_f f^T = Win.T @ xg^T                    psum [f, e]
  wf^T = wt^T * f^T (DVE)                    -> bf16
  PE-transpose wf^T -> wf [e, f] (psum bf16 -> sbuf)
  mm3 per 128-edge tile: conv^T[chunk] += wf_tile.T @ S_tile
       into a [128, 128] psum window per chunk; ~49 flushes to sbuf.
A single manual InstLoadActFuncSet(natural_log_exp_and_others) keeps Exp+Ln
resident: no ACT_TABLE_LOAD thrash (was 2.6 ms in v1).

Tail from sbuf-resident conv^T: z3^T = Wout.T @ conv^T, ssp, v^T = Wd.T @
h^T, y^T = v^T + x^T.  Outputs leave TRANSPOSED [128, NA_PAD]; the host
transposes back.  ssp(x) = Ln(0.5 + 0.5*Exp(x)) exactly.
"""

import math
import sys

import numpy as np
import ml_dtypes

sys.path.insert(0, "/opt/trn_rl_repo")

import concourse.bacc as bacc
import concourse.mybir as mybir
from concourse import tile
from concourse.bass_utils import run_bass_kernel_spmd

dt = mybir.dt
AF = mybir.ActivationFunctionType
BF16 = ml_dtypes.bfloat16

N_CORES = 8
TILE_E = 128            # edges per S tile / conv matmul
BLK = 512               # edges per pipeline block
GRP = 2048              # edges per DMA group (4 blocks, 16 tiles)
CHUNK_A = 128           # atoms per conv psum window
ACT_SET_LN_EXP = 6      # natural_log_exp_and_others in act_info.json


def _ceil(a, b):
    return -(-a // b)


def _to_bf16(a):
    """fp32 -> bf16 with round-to-nearest-even, fast numpy path."""
    a = np.ascontiguousarray(a, dtype=np.float32)
    v = a.view(np.uint32)
    r = ((v + np.uint32(0x7FFF) + ((v >> np.uint32(16)) & np.uint32(1)))
         >> np.uint32(16)).astype(np.uint16)
    return r.view(BF16).reshape(a.shape)


class Plan:
    """Structure constants; identical for every core.  The per-chunk tile
    allotment TPW is fitted to the ACTUAL seg_i data (max over cores), so
    padding is just tile rounding (~3%) and fits by construction."""

    def __init__(self, n_atoms, n_edges, n_in, seg_i):
        assert n_atoms % N_CORES == 0
        self.n_atoms, self.n_edges, self.n_in = n_atoms, n_edges, n_in
        self.NA = n_atoms // N_CORES
        self.NCHUNK_REAL = _ceil(self.NA, CHUNK_A)
        seg_i = np.asarray(seg_i).astype(np.int64)
        bounds = np.searchsorted(seg_i, np.arange(N_CORES + 1) * self.NA)
        mx = np.zeros(self.NCHUNK_REAL, dtype=np.int64)
        for c in range(N_CORES):
            es = seg_i[bounds[c]:bounds[c + 1]] - c * self.NA
            cnt = np.bincount(es // CHUNK_A, minlength=self.NCHUNK_REAL)
            mx = np.maximum(mx, cnt)
        tpw = [int(_ceil(int(m), TILE_E)) for m in mx]
        t_raw = sum(tpw)
        self.T = int(_ceil(t_raw, GRP // TILE_E) * (GRP // TILE_E))
        self.TPW = tpw
        self.E_PC = self.T * TILE_E
        self.NBLK = self.E_PC // BLK
        self.NGRP = self.E_PC // GRP
        self.KC = [min(128, n_in - 128 * i) for i in range(_ceil(n_in, 128))]
        self.NKC = len(self.KC)
        # tile -> chunk map; trailing pad tiles attach to the last chunk
        self.first_tile = []
        self.tile_chunk = []
        for k, n in enumerate(tpw):
            self.first_tile.append(len(self.tile_chunk))
            self.tile_chunk += [k] * n
        self.tile_chunk += [self.NCHUNK_REAL - 1] * (self.T - t_raw)
        self.last_tile = [0] * self.NCHUNK_REAL
        for t, k in enumerate(self.tile_chunk):
            self.last_tile[k] = t
        self.NA_PAD = self.NCHUNK_REAL * CHUNK_A
        self.NSLAB = _ceil(self.NA_PAD, 512)         # tail slabs of 512 atoms
        self.NA_TAIL = self.NSLAB * 512


def shard_inputs(p, x, dijk_bf_T, xgT_all, seg_i):
    """Per-core layout prep. dijk_bf_T/xgT_all carry a zero pad column at
    index n_edges."""
    seg_i = np.asarray(seg_i).astype(np.int64)
    bounds = np.searchsorted(seg_i, np.arange(N_CORES + 1) * p.NA)
    ZCOL = p.n_edges                                  # the zero column

    per_core = []
    for c in range(N_CORES):
        lo, hi = int(bounds[c]), int(bounds[c + 1])
        es = seg_i[lo:hi] - c * p.NA                  # local atoms, sorted
        chunk = es // CHUNK_A

        # per-chunk edge placement into the static tile schedule
        cols = np.full(p.E_PC, ZCOL, dtype=np.int64)  # global edge id or pad
        s_t = np.empty(hi - lo, dtype=np.int64)       # tile of each edge
        s_r = np.empty(hi - lo, dtype=np.int64)       # row within tile
        cnt = np.bincount(chunk, minlength=p.NCHUNK_REAL)
        for k in range(p.NCHUNK_REAL):
            n_k = int(cnt[k])
            if n_k == 0:
                continue
            assert n_k <= p.TPW[k] * TILE_E, (c, k, n_k, p.TPW[k] * TILE_E)
            e0 = int(np.searchsorted(chunk, k))
            base = p.first_tile[k] * TILE_E
            pos = base + np.arange(n_k)
            cols[pos] = lo + e0 + np.arange(n_k)
            s_t[e0:e0 + n_k] = pos // TILE_E
            s_r[e0:e0 + n_k] = pos % TILE_E

        d = dijk_bf_T[:, cols]                        # [n_in, E_PC]
        d0 = np.ascontiguousarray(d[0:128])
        d1 = np.ascontiguousarray(d[128:256])
        d2 = np.ascontiguousarray(d[256:])
        xgT = np.ascontiguousarray(xgT_all[:, cols])  # [128, E_PC]

        S = np.zeros((p.T, TILE_E, CHUNK_A), dtype=BF16)
        s_c = es - chunk * CHUNK_A
        S[s_t, s_r, s_c] = 1.0

        xT = np.zeros((128, p.NA_TAIL), dtype=np.float32)
        xT[:, : p.NA] = x[c * p.NA : (c + 1) * p.NA].T

        per_core.append(dict(d0=d0, d1=d1, d2=d2, xgT=xgT, s_pages=S, xT=xT))
    return per_core


def build_program(p):
    nc = bacc.Bacc(None, target_bir_lowering=False)

    d0 = nc.declare_dram_parameter("d0", [128, p.E_PC], dt.bfloat16, isOutput=False)
    d1 = nc.declare_dram_parameter("d1", [128, p.E_PC], dt.bfloat16, isOutput=False)
    d2 = nc.declare_dram_parameter("d2", [p.KC[2], p.E_PC], dt.bfloat16, isOutput=False)
    xgT = nc.declare_dram_parameter("xgT", [128, p.E_PC], dt.bfloat16, isOutput=False)
    s_pages = nc.declare_dram_parameter(
        "s_pages", [p.T, TILE_E, CHUNK_A], dt.bfloat16, isOutput=False)
    xT = nc.declare_dram_parameter("xT", [128, p.NA_TAIL], dt.float32, isOutput=False)
    w1b = nc.declare_dram_parameter("w1b", [p.n_in, 128], dt.bfloat16, isOutput=False)
    w2b = nc.declare_dram_parameter("w2b", [128, 128], dt.bfloat16, isOutput=False)
    winb = nc.declare_dram_parameter("winb", [128, 128], dt.bfloat16, isOutput=False)
    woutb = nc.declare_dram_parameter("woutb", [128, 128], dt.bfloat16, isOutput=False)
    wdb = nc.declare_dram_parameter("wdb", [128, 128], dt.bfloat16, isOutput=False)
    identb = nc.declare_dram_parameter("identb", [128, 128], dt.bfloat16, isOutput=False)

    y_out = nc.declare_dram_parameter("y_out", [128, p.NA_TAIL], dt.float32, isOutput=True)
    v_out = nc.declare_dram_parameter("v_out", [128, p.NA_TAIL], dt.float32, isOutput=True)

    dsrc = [d0, d1, d2]

    with tile.TileContext(nc) as tc:
        # keep both Exp and Ln tables resident for the whole program
        nc.scalar.add_instruction(
            mybir.InstLoadActFuncSet(
                name=nc.get_next_instruction_name(), ins=[], outs=[],
                act_func_set_id=ACT_SET_LN_EXP,
            )
        )
        with (
            tc.tile_pool(name="const", bufs=1) as constp,
            tc.tile_pool(name="dload", bufs=2) as dload,
            tc.tile_pool(name="stage", bufs=2) as stage,
            tc.tile_pool(name="work", bufs=2) as work,
            tc.tile_pool(name="tail", bufs=2) as tailp,
            tc.tile_pool(name="ps_t1", bufs=2, space="PSUM") as ps_t1,
            tc.tile_pool(name="ps_w", bufs=2, space="PSUM") as ps_w,
            tc.tile_pool(name="ps_f", bufs=1, space="PSUM") as ps_f,
            tc.tile_pool(name="ps_tr", bufs=2, space="PSUM") as ps_tr,
            tc.tile_pool(name="ps_cv", bufs=1, space="PSUM") as ps_cv,
        ):
            # ---- constants ----
            idn = constp.tile([128, 128], dt.bfloat16)
            nc.scalar.dma_start(out=idn[:], in_=identb[:, :])
            half_c = constp.tile([128, 1], dt.float32)
            nc.gpsimd.memset(half_c[:], 0.5)
            nln2 = constp.tile([128, 1], dt.float32)
            nc.gpsimd.memset(nln2[:], -0.6931471805599453)
            w1sb = []
            for kc in range(p.NKC):
                kn = p.KC[kc]
                t = constp.tile([128, 128], dt.bfloat16, name=f"w1sb{kc}")
                nc.scalar.dma_start(out=t[:kn, :], in_=w1b[kc * 128: kc * 128 + kn, :])
                w1sb.append(t)
            w2sb = constp.tile([128, 128], dt.bfloat16)
            nc.scalar.dma_start(out=w2sb[:], in_=w2b[:, :])
            winsb = constp.tile([128, 128], dt.bfloat16)
            nc.scalar.dma_start(out=winsb[:], in_=winb[:, :])
            woutsb = constp.tile([128, 128], dt.bfloat16)
            nc.scalar.dma_start(out=woutsb[:], in_=woutb[:, :])
            wdsb = constp.tile([128, 128], dt.bfloat16)
            nc.scalar.dma_start(out=wdsb[:], in_=wdb[:, :])
            xT_sb = constp.tile([128, p.NA_TAIL], dt.float32)
            nc.gpsimd.dma_start(out=xT_sb[:], in_=xT[:, :])
            convT = constp.tile([128, p.NA_TAIL], dt.bfloat16)

            conv_tiles = {}
            tail_done = set()

            def emit_tail_slab(s):
                # z3^T = Wout.T @ conv^T, ssp, v^T = Wd.T @ h^T, y^T = v + x
                tail_done.add(s)
                a0 = s * 512
                z3 = ps_t1.tile([128, 512], dt.float32, tag="t1", name="z3")
                nc.tensor.matmul(
                    z3[:], woutsb[:], convT[:, a0:a0 + 512],
                    start=True, stop=True)
                e3 = tailp.tile([128, 512], dt.float32, tag="e3")
                nc.scalar.activation(e3[:], z3[:], AF.Exp, bias=nln2[:])
                hT = tailp.tile([128, 512], dt.bfloat16, tag="hT")
                nc.scalar.activation(
                    hT[:], e3[:], AF.Ln, bias=half_c[:])
                vps = ps_w.tile([128, 512], dt.float32, tag="wps", name="vps")
                nc.tensor.matmul(vps[:], wdsb[:], hT[:], start=True, stop=True)
                v_sb = tailp.tile([128, 512], dt.float32, tag="v_sb")
                nc.vector.tensor_copy(v_sb[:], vps[:])
                nc.scalar.dma_start(out=v_out[:, a0:a0 + 512], in_=v_sb[:])
                y_sb = tailp.tile([128, 512], dt.float32, tag="y_sb")
                nc.vector.tensor_tensor(
                    y_sb[:], vps[:], xT_sb[:, a0:a0 + 512],
                    mybir.AluOpType.add)
                nc.scalar.dma_start(out=y_out[:, a0:a0 + 512], in_=y_sb[:])

            # ---- edge pipeline: 3-deep software pipeline ----
            # Iteration `it` emits: loads+phase1(it), phase2(it-1),
            # phase3a(it-2), S-load(it-2), phase3b(it-3).  Every cross-
            # engine dependency gets >= 1 full group of slack, so neither
            # the PE nor the ACT queue head ever waits on fresh results.
            # dijk on the SP HWDGE ring; xgT/S via SWDGE on idle GpSimd.
            t1s_q, wt_q, xg_q, wfT_q, sg_q = {}, {}, {}, {}, {}

            def emit_loads_p1(g):
                e0 = g * GRP
                dg = []
                for kc in range(p.NKC):
                    kn = p.KC[kc]
                    tdg = dload.tile([kn, GRP], dt.bfloat16,
                                     tag=f"dg{kc}", name=f"dg{kc}", bufs=3)
                    nc.sync.dma_start(out=tdg[:], in_=dsrc[kc][:, e0:e0 + GRP])
                    dg.append(tdg)
                xgg = dload.tile([128, GRP], dt.bfloat16, tag="xgg", bufs=4)
                nc.gpsimd.dma_start(out=xgg[:], in_=xgT[:, e0:e0 + GRP])
                xg_q[g] = xgg
                e1g = stage.tile([128, 4, BLK], dt.float32, tag="e1g")
                for b in range(4):
                    t1 = ps_t1.tile([128, BLK], dt.float32, tag="t1")
                    for kc in range(p.NKC):
                        kn = p.KC[kc]
                        nc.tensor.matmul(
                            t1[:], w1sb[kc][:kn, :],
                            dg[kc][:, b * BLK:(b + 1) * BLK],
                            start=(kc == 0), stop=(kc == p.NKC - 1),
                        )
                    nc.scalar.activation(e1g[:, b, :], t1[:], AF.Exp, bias=nln2[:])
                t1sg = stage.tile([128, 4, BLK], dt.bfloat16, tag="t1sg",
                                  bufs=3)
                nc.scalar.activation(
                    t1sg[:], e1g[:], AF.Ln, bias=half_c[:])
                t1s_q[g] = t1sg

            def emit_phase2(g):
                t1sg = t1s_q.pop(g)
                ewg = stage.tile([128, 4, BLK], dt.float32, tag="ewg")
                for b in range(4):
                    wps = ps_w.tile([128, BLK], dt.float32, tag="wps")
                    nc.tensor.matmul(
                        wps[:], w2sb[:], t1sg[:, b, :], start=True, stop=True)
                    nc.scalar.activation(ewg[:, b, :], wps[:], AF.Exp, bias=nln2[:])
                wtg = stage.tile([128, 4, BLK], dt.float32, tag="wtg",
                                 bufs=3)
                nc.scalar.activation(
                    wtg[:], ewg[:], AF.Ln, bias=half_c[:])
                wt_q[g] = wtg

            def emit_phase3a(g):
                wtg = wt_q.pop(g)
                xgg = xg_q.pop(g)
                wfT_g = []
                for b in range(4):
                    fps = ps_f.tile([128, BLK], dt.float32, tag="fps")
                    nc.tensor.matmul(
                        fps[:], winsb[:], xgg[:, b * BLK:(b + 1) * BLK],
                        start=True, stop=True)
                    wfT = work.tile([128, BLK], dt.bfloat16, tag="wfT",
                                    bufs=8)
                    nc.vector.tensor_tensor(
                        wfT[:], wtg[:, b, :], fps[:], mybir.AluOpType.mult)
                    wfT_g.append(wfT)
                wfT_q[g] = wfT_g

            def emit_sg_load(g):
                sg = dload.tile([128, 16, CHUNK_A], dt.bfloat16, tag="sg")
                nc.gpsimd.dma_start(
                    out=sg[:],
                    in_=s_pages[g * 16:(g + 1) * 16, :, :].rearrange(
                        "t pp c -> pp t c", pp=128),
                )
                sg_q[g] = sg

            def emit_phase3b(g):
                wfT_list = wfT_q.pop(g)
                sgq = sg_q.pop(g)
                for b in range(4):
                    wfT = wfT_list[b]
                    wfP = ps_tr.tile([128, BLK], dt.bfloat16, tag="wfP",
                                     name="wfP")
                    for i in range(4):
                        nc.tensor.transpose(
                            wfP[:, i * 128:(i + 1) * 128],
                            wfT[:, i * 128:(i + 1) * 128], idn[:])
                    wf = work.tile([128, BLK], dt.bfloat16, tag="wf",
                                   name="wf")
                    nc.vector.tensor_copy(wf[:], wfP[:])
                    for i in range(4):
                        t = g * 16 + b * 4 + i           # global tile id
                        k = p.tile_chunk[t]
                        if p.first_tile[k] == t:
                            cv = ps_cv.tile([128, CHUNK_A], dt.float32,
                                            tag="cv", name="cv")
                            nc.vector.memset(cv[:], 0.0)
                            conv_tiles[k] = cv
                        cv = conv_tiles[k]
                        nc.tensor.matmul(
                            cv[:], wf[:, i * 128:(i + 1) * 128],
                            sgq[:, b * 4 + i, :],
                            start=False, stop=(p.last_tile[k] == t),
                            skip_group_check=True,
                        )
                        if p.last_tile[k] == t:
                            nc.vector.tensor_copy(
                                convT[:, k * CHUNK_A:(k + 1) * CHUNK_A],
                                cv[:])
                            del conv_tiles[k]
                            # interleave tail slabs whose conv chunks are
                            # all flushed (flushes are in chunk order)
                            for s in range(p.NSLAB):
                                if (s not in tail_done
                                        and min(4 * s + 3,
                                                p.NCHUNK_REAL - 1) <= k):
                                    emit_tail_slab(s)

            for it in range(p.NGRP + 3):
                if it < p.NGRP:
                    emit_loads_p1(it)
                if 1 <= it < p.NGRP + 1:
                    emit_phase2(it - 1)
                if 2 <= it < p.NGRP + 2:
                    emit_sg_load(it - 2)
                    emit_phase3a(it - 2)
                if 3 <= it < p.NGRP + 3:
                    emit_phase3b(it - 3)

            # ---- tail safety sweep (slabs not emitted inline) ----
            for s in range(p.NSLAB):
                if s not in tail_done:
                    emit_tail_slab(s)

    nc.finalize()
    return nc


_PROG_CACHE = {}


def kernel(x, dijk, W1, b1, W2, b2, Win, Wout, bout, Wd, bd, idx_j, seg_i, seg_j):
    x = np.ascontiguousarray(np.asarray(x, dtype=np.float32))
    dijk = np.ascontiguousarray(np.asarray(dijk, dtype=np.float32))
    for b in (b1, b2, bout, bd):
        assert np.abs(np.asarray(b)).max() == 0.0, "nonzero biases unsupported"

    n_atoms, n_basis = x.shape
    n_edges, n_in = dijk.shape
    assert n_basis == 128 and np.asarray(W2).shape == (128, 128)

    p = Plan(n_atoms, n_edges, n_in, seg_i)

    # global host-side layout transforms (shared across cores)
    dijk_bf_T = np.zeros((n_in, n_edges + 1), dtype=BF16)
    dijk_bf_T[:, :n_edges] = _to_bf16(dijk).T
    x_bf = _to_bf16(x)
    idx = np.asarray(idx_j).astype(np.int64)
    xgT_all = np.zeros((128, n_edges + 1), dtype=BF16)
    xgT_all[:, :n_edges] = x_bf[idx].T

    per_core = shard_inputs(p, x, dijk_bf_T, xgT_all, seg_i)
    del dijk_bf_T, xgT_all

    key = (n_atoms, n_edges, n_in, tuple(p.TPW))
    if key not in _PROG_CACHE:
        _PROG_CACHE[key] = build_program(p)
    nc = _PROG_CACHE[key]

    common = dict(
        w1b=_to_bf16(np.asarray(W1, dtype=np.float32)),
        w2b=_to_bf16(np.asarray(W2, dtype=np.float32)),
        winb=_to_bf16(np.asarray(Win, dtype=np.float32)),
        woutb=_to_bf16(np.asarray(Wout, dtype=np.float32)),
        wdb=_to_bf16(np.asarray(Wd, dtype=np.float32)),
        identb=_to_bf16(np.eye(128, dtype=np.float32)),
    )
    in_maps = [{**common, **pc} for pc in per_core]
    res = run_bass_kernel_spmd(nc, in_maps, list(range(N_CORES)))
    global LAST_RESULTS
    LAST_RESULTS = res

    y = np.empty((n_atoms, 128), dtype=np.float32)
    v = np.empty((n_atoms, 128), dtype=np.float32)
    for c in range(N_CORES):
        y[c * p.NA:(c + 1) * p.NA] = res.results[c]["y_out"][:, : p.NA].T
        v[c * p.NA:(c + 1) * p.NA] = res.results[c]["v_out"][:, : p.NA].T
    return (y, v)
